# revision 3
# baseline (speedup 1.0000x reference)
"""Causal self-attention with RoPE on 8 TRN2 NeuronCores.

Sharding: 2 (batch) x 4 (head-group tensor parallel). Core c handles
batch b=c//4 and heads [4g, 4g+4) with g=c%4. Each core computes its
q,k,v projections, RoPE, causal attention (transposed-scores flash
layout), and its partial of the output projection; the host sums the
4 partials per batch (the "all-reduce").

v3: software-pipelined emission. QKV(tb+1) and proj(tb-1) matmul
groups are interleaved into attention(tb)'s kt loop so the PE never
idles across t-block boundaries (keeps HAM at 8/8); causal mask is
applied as an additive identity-matmul into the score PSUM group
(instead of DVE multiplies); RoPE for block tb+1 is deferred into the
PSUM banks freed by norm(tb); inputs are host-pre-tiled so each
tensor loads with one or two large DMAs, issued from multiple engines.

Self-contained: hardcodes shapes from the problem spec.
"""
import numpy as np
import ml_dtypes

import concourse.bass as bass
import concourse.mybir as mybir
import concourse.tile as tile
from concourse import bacc
from concourse.bass_utils import run_bass_kernel_spmd

F32 = mybir.dt.float32
BF16 = mybir.dt.bfloat16

B, T, DIM = 2, 2048, 1024
HEADS, HEAD_DIM = 16, 64
INNER = HEADS * HEAD_DIM
ROPE_BASE = 10000.0
N_CORES = 8
TPG = 4                      # tensor-parallel group size (head groups)
HPC = HEADS // TPG           # heads per core = 4
LOC = HPC * HEAD_DIM         # local inner = 256
SCALE = 1.0 / np.sqrt(HEAD_DIM)
MASK_NEG = -30000.0          # additive causal mask (pre-scale)

TB = 512                     # t block for QKV / q block for attention
NTB = T // TB                # 4
ND = DIM // 128              # 8 contraction chunks


def _host_constants():
    inv_freq = 1.0 / (ROPE_BASE ** (np.arange(0, HEAD_DIM, 2, dtype=np.float32) / HEAD_DIM))
    t = np.arange(T, dtype=np.float32)
    freqs = np.outer(t, inv_freq).astype(np.float32)          # [T, 32]
    cos32 = np.cos(freqs).T.astype(np.float32)                # [32, T]
    sin32 = np.sin(freqs).T.astype(np.float32)
    cosT = np.tile(cos32, (4, 1))                             # [128, T]
    sinT = np.tile(sin32, (4, 1))

    # rot matrix: rot[m] = -x[m+32] (m%64<32), +x[m-32] (m%64>=32); lhsT[k, m]
    prot = np.zeros((128, 128), dtype=np.float32)
    for blk in range(2):
        o = blk * 64
        for m in range(32):
            prot[o + m + 32, o + m] = -1.0
            prot[o + m, o + m + 32] = 1.0

    # additive causal mask for the diagonal 128-col block: 0 where j >= p
    j = np.arange(128)[None, :]
    p = np.arange(128)[:, None]
    mask1 = np.where(j >= p, 0.0, MASK_NEG).astype(np.float32)   # [128, 128]
    mask2 = np.concatenate([mask1, mask1], axis=1)               # [128, 256] (both heads)
    ident = np.eye(128, dtype=np.float32)
    return cosT, sinT, prot, mask2, ident


def build_kernel(tc):
    nc = tc.nc
    # host-pre-tiled dram layouts (all 2D, partition-major tiles along free)
    x_til = nc.dram_tensor("x_til", [128, NTB * ND * TB], BF16, kind="ExternalInput").ap()
    wqk0_d = nc.dram_tensor("wqk0", [128, 2 * LOC], BF16, kind="ExternalInput").ap()
    wqkR_d = nc.dram_tensor("wqkR", [128, (ND - 1) * 2 * LOC], BF16, kind="ExternalInput").ap()
    wv_d = nc.dram_tensor("wv_til", [128, ND * LOC], BF16, kind="ExternalInput").ap()
    wpr_d = nc.dram_tensor("wpr_til", [128, 2 * DIM], BF16, kind="ExternalInput").ap()
    cosT_d = nc.dram_tensor("cosT", [128, T], BF16, kind="ExternalInput").ap()
    sinT_d = nc.dram_tensor("sinT", [128, T], BF16, kind="ExternalInput").ap()
    prot_d = nc.dram_tensor("prot", [128, 128], BF16, kind="ExternalInput").ap()
    mask2_d = nc.dram_tensor("mask2", [128, 2 * 128], BF16, kind="ExternalInput").ap()
    ident_d = nc.dram_tensor("ident", [128, 128], BF16, kind="ExternalInput").ap()
    out_d = nc.dram_tensor("out", [128, NTB * 4 * DIM], BF16, kind="ExternalOutput").ap()

    with (
        tc.tile_pool(name="const", bufs=1) as const,
        tc.tile_pool(name="xt", bufs=2) as xt_pool,
        tc.tile_pool(name="persist", bufs=1) as persist,
        tc.tile_pool(name="work", bufs=4) as work,
        tc.tile_pool(name="expp", bufs=6) as expp,
        tc.tile_pool(name="ps_sc", bufs=2, space="PSUM") as ps_sc,
        tc.tile_pool(name="ps_acc", bufs=1, space="PSUM") as ps_acc,
        tc.tile_pool(name="ps_mm", bufs=1, space="PSUM") as ps_mm,
    ):
        # ---- input DMAs: sync engine carries the startup-critical x/wqk;
        # gpsimd issues the constants in parallel ----
        xt0a = const.tile([128, TB], BF16, tag="xt0a")
        nc.sync.dma_start(out=xt0a, in_=x_til[:, 0:TB])
        wqk0 = const.tile([128, 2 * LOC], BF16, tag="wqk0")
        nc.sync.dma_start(out=wqk0, in_=wqk0_d)
        wqkR = const.tile([128, (ND - 1) * 2 * LOC], BF16, tag="wqkR")
        nc.sync.dma_start(out=wqkR, in_=wqkR_d)
        xt0b = const.tile([128, (ND - 1) * TB], BF16, tag="xt0b")
        nc.sync.dma_start(out=xt0b, in_=x_til[:, TB:ND * TB])

        cos_sb = const.tile([128, T], BF16, tag="cos")
        nc.gpsimd.dma_start(out=cos_sb, in_=cosT_d)
        sin_sb = const.tile([128, T], BF16, tag="sin")
        nc.gpsimd.dma_start(out=sin_sb, in_=sinT_d)
        prot_sb = const.tile([128, 128], BF16, tag="prot")
        nc.gpsimd.dma_start(out=prot_sb, in_=prot_d)
        mask2_sb = const.tile([128, 2, 128], BF16, tag="mask2")
        nc.gpsimd.dma_start(out=mask2_sb.rearrange("p a b -> p (a b)"), in_=mask2_d)
        ident_sb = const.tile([128, 128], BF16, tag="ident")
        nc.gpsimd.dma_start(out=ident_sb, in_=ident_d)
        wv_sb = const.tile([128, ND, LOC], BF16, tag="wv")
        nc.gpsimd.dma_start(out=wv_sb.rearrange("p a b -> p (a b)"), in_=wv_d)
        wpr_sb = const.tile([128, 2, DIM], BF16, tag="wpr")
        nc.gpsimd.dma_start(out=wpr_sb.rearrange("p a b -> p (a b)"), in_=wpr_d)

        ones_sb = const.tile([128, 1], BF16, tag="ones")
        nc.vector.memset(ones_sb, 1.0)
        ones2_sb = const.tile([128, 64], BF16, tag="ones2")
        nc.vector.memset(ones2_sb, 1.0)

        # x prefetch for tb=1 right behind the startup DMAs
        xt_sb = {}
        x1 = xt_pool.tile([128, ND, TB], BF16, tag="x", name="x_1")
        nc.sync.dma_start(out=x1.rearrange("p a b -> p (a b)"),
                          in_=x_til[:, ND * TB:2 * ND * TB])
        xt_sb[1] = x1

        def x_chunk(tb, d):
            if tb == 0:
                return xt0a if d == 0 else xt0b[:, (d - 1) * TB:d * TB]
            return xt_sb[tb][:, d, :]

        def wqk_chunk(d, m):
            # columns m*128:(m+1)*128 of the d-th 128-row chunk of w_qk
            if d == 0:
                return wqk0[:, m * 128:(m + 1) * 128]
            return wqkR[:, (d - 1) * 2 * LOC + m * 128:(d - 1) * 2 * LOC + (m + 1) * 128]

        # persistent per-phase outputs
        qk_rope = [[persist.tile([128, TB], BF16, tag=f"qkr{m}_{tb}", name=f"qkr{m}_{tb}")
                    for tb in range(NTB)] for m in range(4)]
        v_sb = [persist.tile([128, LOC], BF16, tag=f"v{ts}", name=f"v{ts}")
                for ts in range(4 * NTB)]
        raw_sb = {}       # (tb, m) -> raw q/k pair tile awaiting rope
        outT_sb = {}      # (qb, p)

        acc_tags = ["mm", "av0", "av1", "rsum"]

        # ---------- emission helpers ----------
        def emit_qk_group(tb, m, tag="mm"):
            ps = (ps_mm if tag == "mm" else ps_acc).tile(
                [128, TB], F32, tag=tag, name=f"qk1_{m}_{tb}")
            for d in range(ND):
                nc.tensor.matmul(ps, lhsT=wqk_chunk(d, m), rhs=x_chunk(tb, d),
                                 start=(d == 0), stop=(d == ND - 1))
            r = work.tile([128, TB], BF16, tag=f"raw{m}", name=f"raw{m}_{tb}")
            nc.vector.tensor_copy(r, ps)
            raw_sb[(tb, m)] = r

        def emit_v_group(tb, s, tag="mm"):
            ts = tb * 4 + s
            ps = (ps_mm if tag == "mm" else ps_acc).tile(
                [128, LOC], F32, tag=tag, name=f"v_ps{ts}")
            for d in range(ND):
                nc.tensor.matmul(ps, lhsT=x_chunk(tb, d)[:, s * 128:(s + 1) * 128],
                                 rhs=wv_sb[:, d, :],
                                 start=(d == 0), stop=(d == ND - 1))
            nc.vector.tensor_copy(v_sb[ts], ps)

        def emit_rope(tb, m, tag):
            rot = (ps_mm if tag == "mm" else ps_acc).tile(
                [128, TB], F32, tag=tag, name=f"rot_{m}_{tb}")
            r = raw_sb.pop((tb, m))
            nc.tensor.matmul(rot, lhsT=prot_sb, rhs=r, start=True, stop=True)
            qc = work.tile([128, TB], BF16, tag="qc")
            nc.vector.tensor_mul(qc, r, cos_sb[:, tb * TB:(tb + 1) * TB])
            rs = work.tile([128, TB], BF16, tag="rs")
            nc.vector.tensor_mul(rs, rot, sin_sb[:, tb * TB:(tb + 1) * TB])
            nc.vector.tensor_add(qk_rope[m][tb], qc, rs)

        def emit_proj(qb, s, n):
            ps = ps_mm.tile([128, TB], F32, tag="mm", name=f"pr{qb}_{s}_{n}")
            for p in range(2):
                nc.tensor.matmul(ps, lhsT=outT_sb[(qb, p)][:, s * 128:(s + 1) * 128],
                                 rhs=wpr_sb[:, p, n * TB:(n + 1) * TB],
                                 start=(p == 0), stop=(p == 1))
            pr = work.tile([128, TB], BF16, tag="pr_sb")
            nc.vector.tensor_copy(pr, ps)
            nc.gpsimd.dma_start(
                out=out_d[:, (qb * 4 + s) * DIM + n * TB:(qb * 4 + s) * DIM + (n + 1) * TB],
                in_=pr)

        # ---------- pre-loop: QKV(0) + rope(0) ----------
        for m in range(4):
            emit_qk_group(0, m, acc_tags[m])
        for s in range(4):
            emit_v_group(0, s, acc_tags[s])
        for m in range(4):
            emit_rope(0, m, acc_tags[m])

        # ---------- main pipelined loop ----------
        for tb in range(NTB):
            qb = tb
            nkt = 4 * (qb + 1)
            # prefetch x for tb+2 (consumed by QKV(tb+2) during attention(tb+1))
            if tb + 2 < NTB:
                xn = xt_pool.tile([128, ND, TB], BF16, tag="x", name=f"x_{tb + 2}")
                nc.sync.dma_start(
                    out=xn.rearrange("p a b -> p (a b)"),
                    in_=x_til[:, (tb + 2) * ND * TB:(tb + 3) * ND * TB])
                xt_sb[tb + 2] = xn

            # interleave units: QKV(tb+1) groups + proj(tb-1) pieces
            units = []
            if tb + 1 < NTB:
                units += [lambda m=m: emit_qk_group(tb + 1, m) for m in range(4)]
                units += [lambda s=s: emit_v_group(tb + 1, s) for s in range(4)]
            if tb - 1 >= 0:
                units += [lambda s=s, n=n: emit_proj(tb - 1, s, n)
                          for s in range(4) for n in range(2)]
            nu = len(units)
            ui = 0

            av_ps = [ps_acc.tile([128, TB], F32, tag=f"av{p}", name=f"av{p}_{qb}")
                     for p in range(2)]
            rsum_ps = ps_acc.tile([128, TB], F32, tag="rsum", name=f"rsum_{qb}")
            for kt in range(nkt):
                ktl = kt - 4 * qb
                a = 128 * ktl if ktl >= 0 else 0
                w = TB - a
                tbk, ok = kt // 4, (kt % 4) * 128
                for p in range(2):
                    sc2 = ps_sc.tile([128, 2, TB], F32, tag="sc", name=f"sc{qb}_{kt}_{p}")
                    for j in range(2):
                        nc.tensor.matmul(
                            sc2[:, j, 0:w],
                            lhsT=qk_rope[2 + p][tbk][64 * j:64 * j + 64, ok:ok + 128],
                            rhs=qk_rope[p][qb][64 * j:64 * j + 64, a:TB],
                            start=True, stop=(ktl < 0), tile_position=(64 * j, 0),
                        )
                    if ktl >= 0:
                        # additive causal mask on the diagonal 128-col block
                        nc.tensor.matmul(
                            sc2[:, :, 0:128], lhsT=ident_sb, rhs=mask2_sb,
                            start=False, stop=True, skip_group_check=True,
                        )
                    exp2 = expp.tile([128, 2, TB], BF16, tag="exp", name=f"exp{qb}_{kt}_{p}")
                    nc.scalar.activation(exp2[:, :, 0:w], sc2[:, :, 0:w],
                                         mybir.ActivationFunctionType.Exp,
                                         scale=float(SCALE))
                    for j in range(2):
                        h = 2 * p + j
                        nc.tensor.matmul(
                            av_ps[p][64 * j:64 * j + 64, a:TB],
                            lhsT=v_sb[kt][:, 64 * h:64 * h + 64],
                            rhs=exp2[:, j, 0:w],
                            start=(kt == 0), stop=(kt == nkt - 1),
                            skip_group_check=True,
                            tile_position=(0, 64 * j),
                        )
                    for j in range(2):
                        h = 2 * p + j
                        nc.tensor.matmul(
                            rsum_ps[32 * h:32 * h + 1, a:TB],
                            lhsT=ones_sb,
                            rhs=exp2[:, j, 0:w],
                            start=(kt == 0), stop=(kt == nkt - 1),
                            skip_group_check=True,
                            tile_position=(0, 32 * h),
                        )
                # drain interleave units evenly across the kt loop
                want = (kt + 1) * nu // nkt
                while ui < want:
                    units[ui]()
                    ui += 1

            # softmax normalizer: copy rowsums to SBUF, replicate via K=1
            # outer-product matmuls, then approx reciprocal on the dense tile
            rsum_sb = work.tile([128, TB], BF16, tag="recip")
            nc.vector.tensor_copy(rsum_sb, rsum_ps)
            bc2 = ps_sc.tile([128, 2, TB], F32, tag="sc", name=f"bc{qb}")
            for p in range(2):
                for j in range(2):
                    h = 2 * p + j
                    nc.tensor.matmul(
                        bc2[64 * j:64 * j + 64, p, :],
                        lhsT=ones2_sb[32 * h:32 * h + 1, :],
                        rhs=rsum_sb[32 * h:32 * h + 1, :],
                        start=True, stop=True, skip_group_check=True,
                        tile_position=(32 * h, 64 * j),
                    )
            recip2_sb = work.tile([128, 2, TB], F32, tag="recipb")
            nc.vector.reciprocal_approx_fast(out=recip2_sb, in_=bc2)
            for p in range(2):
                o_t = persist.tile([128, TB], BF16, tag=f"outT{qb}_{p}", name=f"outT{qb}_{p}")
                nc.vector.tensor_mul(o_t, av_ps[p], recip2_sb[:, p, :])
                outT_sb[(qb, p)] = o_t

            # deferred rope for the next block, into the freed accumulator banks
            if tb + 1 < NTB:
                for m in range(4):
                    emit_rope(tb + 1, m, acc_tags[(m + 1) % 4])

        # ---------- tail: final block's output projection ----------
        for s in range(4):
            for n in range(2):
                emit_proj(NTB - 1, s, n)


def shard_inputs(x, w_qkv, w_proj):
    """Full inputs -> list of 8 per-core input maps (pre-tiled layouts)."""
    cosT, sinT, prot, mask2, ident = _host_constants()
    x = np.ascontiguousarray(np.asarray(x, dtype=np.float32))
    w_qkv = np.asarray(w_qkv, dtype=np.float32)
    w_proj = np.asarray(w_proj, dtype=np.float32)
    bf = ml_dtypes.bfloat16
    in_maps = []
    for c in range(N_CORES):
        b, g = c // TPG, c % TPG
        xT = x[b].T                                           # [DIM, T]
        # x_til[p, ((tb*ND + d)*TB + j)] = xT[d*128+p, tb*TB+j]
        xt4 = xT.reshape(ND, 128, NTB, TB)                    # [d, p, tb, j]
        x_til = np.ascontiguousarray(xt4.transpose(1, 2, 0, 3).reshape(128, NTB * ND * TB))
        wq = w_qkv[:, g * LOC:(g + 1) * LOC]
        wk = w_qkv[:, INNER + g * LOC:INNER + (g + 1) * LOC]
        wv = w_qkv[:, 2 * INNER + g * LOC:2 * INNER + (g + 1) * LOC]
        w_qk = np.concatenate([wq, wk], axis=1)               # [DIM, 512]
        wqk_t = w_qk.reshape(ND, 128, 2 * LOC)                # [d, p, c]
        wqk0 = np.ascontiguousarray(wqk_t[0])
        wqkR = np.ascontiguousarray(wqk_t[1:].transpose(1, 0, 2).reshape(128, (ND - 1) * 2 * LOC))
        wv_t = np.ascontiguousarray(
            wv.reshape(ND, 128, LOC).transpose(1, 0, 2).reshape(128, ND * LOC))
        w_pr = w_proj[g * LOC:(g + 1) * LOC, :]               # [256, DIM]
        wpr_t = np.ascontiguousarray(
            w_pr.reshape(2, 128, DIM).transpose(1, 0, 2).reshape(128, 2 * DIM))
        in_maps.append({
            "x_til": x_til.astype(bf),
            "wqk0": wqk0.astype(bf),
            "wqkR": wqkR.astype(bf),
            "wv_til": wv_t.astype(bf),
            "wpr_til": wpr_t.astype(bf),
            "cosT": cosT.astype(bf),
            "sinT": sinT.astype(bf),
            "prot": prot.astype(bf),
            "mask2": mask2.astype(bf),
            "ident": ident.astype(bf),
        })
    return in_maps


_CACHE = {}


def _get_compiled():
    if "nc" not in _CACHE:
        nc = bacc.Bacc("TRN2", target_bir_lowering=False, debug=False,
                       enable_asserts=True, num_devices=N_CORES)
        with tile.TileContext(nc) as tc:
            build_kernel(tc)
        nc.compile()
        _CACHE["nc"] = nc
    return _CACHE["nc"]


def kernel(x, w_qkv, w_proj):
    nc = _get_compiled()
    in_maps = shard_inputs(x, w_qkv, w_proj)
    res = run_bass_kernel_spmd(nc, in_maps, core_ids=list(range(N_CORES)))
    # out tile [128, 16*1024]: rows = t within 128-subtile, free = (qb*4+s, dim)
    outs = []
    for c in range(N_CORES):
        o = np.asarray(res.results[c]["out"], dtype=np.float32)
        o = o.reshape(128, 16, DIM).transpose(1, 0, 2).reshape(T, DIM)
        outs.append(o)
    full = np.stack([
        np.sum([outs[b * TPG + g] for g in range(TPG)], axis=0, dtype=np.float32)
        for b in range(B)
    ])
    return full.astype(np.float32)


# revision 8
# speedup vs baseline: 1.0745x; 1.0745x over previous
"""Causal self-attention with RoPE on 8 TRN2 NeuronCores.

Sharding: 2 (batch) x 4 (head-group tensor parallel). Core c handles
batch b=c//4 and heads [4g, 4g+4) with g=c%4. Each core computes its
q,k,v projections, RoPE, causal attention (transposed-scores flash
layout), and its partial of the output projection; the host sums the
4 partials per batch (the "all-reduce").

v4: one-stage-pipelined attention (av/rowsum trail scores/exp by one
kt so the in-order PE queue never blocks on the exp semaphore); RoPE
rotation via DVE stream_shuffle with the sign folded into the sin
table (no PE rotation matmuls, no rope PSUM); QKV(tb+1)/proj(tb-1)
emitted as half-PSUM-bank ping-pong units inside attention(tb)'s kt
loop; startup DMAs split per 128-row chunk and issued from four
engine queues in parallel.

Self-contained: hardcodes shapes from the problem spec.
"""
import numpy as np
import ml_dtypes

import concourse.bass as bass
import concourse.mybir as mybir
import concourse.tile as tile
from concourse import bacc
from concourse.bass_utils import run_bass_kernel_spmd

F32 = mybir.dt.float32
BF16 = mybir.dt.bfloat16

B, T, DIM = 2, 2048, 1024
HEADS, HEAD_DIM = 16, 64
INNER = HEADS * HEAD_DIM
ROPE_BASE = 10000.0
N_CORES = 8
TPG = 4                      # tensor-parallel group size (head groups)
HPC = HEADS // TPG           # heads per core = 4
LOC = HPC * HEAD_DIM         # local inner = 256
SCALE = 1.0 / np.sqrt(HEAD_DIM)

TB = 512                     # t block for QKV / q block for attention
NTB = T // TB                # 4
ND = DIM // 128              # 8 contraction chunks
HB = 256                     # half-bank column count for ping-pong units

# stream_shuffle permutes within 32-partition blocks (mask replicated
# across blocks). We reorder each head's 64 q/k dims as
# [0..15, 32..47, 16..31, 48..63] so the rope partner (d <-> d+32) sits
# 16 partitions away inside the same 32-block; the shuffle is then a
# 16-half swap. Scores are invariant to this (same perm on q and k).
SHUF = list(range(16, 32)) + list(range(16))
PERM64 = list(range(16)) + list(range(32, 48)) + list(range(16, 32)) + list(range(48, 64))


def _host_constants():
    inv_freq = 1.0 / (ROPE_BASE ** (np.arange(0, HEAD_DIM, 2, dtype=np.float32) / HEAD_DIM))
    t = np.arange(T, dtype=np.float32)
    freqs = np.outer(t, inv_freq).astype(np.float32)          # [T, 32]
    cos32 = np.cos(freqs).T.astype(np.float32)                # [32, T]
    sin32 = np.sin(freqs).T.astype(np.float32)
    cos64 = np.tile(cos32, (2, 1))                            # [64, T]
    sin64 = np.tile(sin32, (2, 1))
    perm = np.array(PERM64)
    # per-head permuted tables; rotate-half sign folded into sin
    cosP = cos64[perm]                                        # [64, T]
    sgn = np.where(perm < 32, -1.0, 1.0)[:, None]
    sinP = sin64[perm] * sgn
    cosT = np.tile(cosP, (2, 1))                              # [128, T]
    sinT2 = np.tile(sinP, (2, 1))

    # post-exp 0/1 causal mask for the diagonal 128-col block: keep j >= p
    j = np.arange(128)[None, :]
    p = np.arange(128)[:, None]
    mask01 = (j >= p).astype(np.float32)                      # [128, 128]
    return cosT, sinT2, mask01


def build_kernel(tc):
    nc = tc.nc
    x_til = nc.dram_tensor("x_til", [128, NTB * ND * TB], BF16, kind="ExternalInput").ap()
    wqk0_d = nc.dram_tensor("wqk0", [128, 2 * LOC], BF16, kind="ExternalInput").ap()
    wqkR_d = nc.dram_tensor("wqkR", [128, (ND - 1) * 2 * LOC], BF16, kind="ExternalInput").ap()
    wv_d = nc.dram_tensor("wv_til", [128, ND * LOC], BF16, kind="ExternalInput").ap()
    wpr_d = nc.dram_tensor("wpr_til", [128, 2 * DIM], BF16, kind="ExternalInput").ap()
    cosT_d = nc.dram_tensor("cosT", [128, T], BF16, kind="ExternalInput").ap()
    sinT_d = nc.dram_tensor("sinT2", [128, T], BF16, kind="ExternalInput").ap()
    mask_d = nc.dram_tensor("mask01", [128, 128], BF16, kind="ExternalInput").ap()
    out_d = nc.dram_tensor("out", [128, NTB * 4 * DIM], BF16, kind="ExternalOutput").ap()

    with (
        tc.tile_pool(name="const", bufs=1) as const,
        tc.tile_pool(name="xt", bufs=2) as xt_pool,
        tc.tile_pool(name="persist", bufs=1) as persist,
        tc.tile_pool(name="work", bufs=4) as work,
        tc.tile_pool(name="prp", bufs=2) as prp,
        tc.tile_pool(name="expp", bufs=6) as expp,
        tc.tile_pool(name="ps_sc", bufs=2, space="PSUM") as ps_sc,
        tc.tile_pool(name="ps_acc", bufs=1, space="PSUM") as ps_acc,
        tc.tile_pool(name="ps_mm", bufs=1, space="PSUM") as ps_mm,
    ):
        # ---- startup DMAs: x(0)/w_qk chunks split per d, spread across
        # sync + vector queues; constants on scalar/gpsimd queues ----
        xt0a = const.tile([128, TB], BF16, tag="xt0a")
        nc.sync.dma_start(out=xt0a, in_=x_til[:, 0:TB])
        wqk0 = const.tile([128, 2 * LOC], BF16, tag="wqk0")
        nc.sync.dma_start(out=wqk0, in_=wqk0_d)
        wqkR = const.tile([128, (ND - 1) * 2 * LOC], BF16, tag="wqkR")
        xt0b = const.tile([128, (ND - 1) * TB], BF16, tag="xt0b")
        for d in range(1, ND):
            eng = nc.sync if d <= 4 else nc.scalar
            eng.dma_start(out=wqkR[:, (d - 1) * 2 * LOC:d * 2 * LOC],
                          in_=wqkR_d[:, (d - 1) * 2 * LOC:d * 2 * LOC])
            eng.dma_start(out=xt0b[:, (d - 1) * TB:d * TB],
                          in_=x_til[:, d * TB:(d + 1) * TB])

        cos_sb = const.tile([128, T], BF16, tag="cos")
        nc.scalar.dma_start(out=cos_sb, in_=cosT_d)
        sin_sb = const.tile([128, T], BF16, tag="sin")
        nc.scalar.dma_start(out=sin_sb, in_=sinT_d)
        mask_sb = const.tile([128, 128], BF16, tag="mask")
        nc.gpsimd.dma_start(out=mask_sb, in_=mask_d)
        wv_sb = const.tile([128, ND, LOC], BF16, tag="wv")
        nc.gpsimd.dma_start(out=wv_sb.rearrange("p a b -> p (a b)"), in_=wv_d)
        wpr_sb = const.tile([128, 2, DIM], BF16, tag="wpr")
        nc.gpsimd.dma_start(out=wpr_sb.rearrange("p a b -> p (a b)"), in_=wpr_d)

        ones_sb = const.tile([128, 1], BF16, tag="ones")
        nc.vector.memset(ones_sb, 1.0)
        ones2_sb = const.tile([128, 64], BF16, tag="ones2")
        nc.vector.memset(ones2_sb, 1.0)

        xt_sb = {}
        x1 = xt_pool.tile([128, ND, TB], BF16, tag="x", name="x_1")
        nc.gpsimd.dma_start(out=x1.rearrange("p a b -> p (a b)"),
                            in_=x_til[:, ND * TB:2 * ND * TB])
        xt_sb[1] = x1

        mask_bc = mask_sb.rearrange("p (o n) -> p o n", o=1).to_broadcast([128, 2, 128])

        def x_chunk(tb, d):
            if tb == 0:
                return xt0a if d == 0 else xt0b[:, (d - 1) * TB:d * TB]
            return xt_sb[tb][:, d, :]

        def wqk_chunk(d, m):
            if d == 0:
                return wqk0[:, m * 128:(m + 1) * 128]
            return wqkR[:, (d - 1) * 2 * LOC + m * 128:(d - 1) * 2 * LOC + (m + 1) * 128]

        # persistent per-phase outputs
        qk_rope = [[persist.tile([128, TB], BF16, tag=f"qkr{m}_{tb}", name=f"qkr{m}_{tb}")
                    for tb in range(NTB)] for m in range(4)]
        v_sb = [persist.tile([128, LOC], BF16, tag=f"v{ts}", name=f"v{ts}")
                for ts in range(4 * NTB)]
        raw_sb = {}
        outT_sb = {}
        pr_tiles = {}

        # the single ping-pong PSUM bank for pipelined QKV/v/proj units
        mm2 = ps_mm.tile([128, 2, HB], F32, tag="mm", name="mm2")
        half = [0]

        def next_half():
            h = half[0]
            half[0] ^= 1
            return h

        # ---------- emission units ----------
        def emit_qk_half(tb, m, c):
            """c in {0,1}: column half of the [128, TB] q/k pair output."""
            h = next_half()
            ps = mm2[:, h, :]
            for d in range(ND):
                nc.tensor.matmul(ps, lhsT=wqk_chunk(d, m),
                                 rhs=x_chunk(tb, d)[:, c * HB:(c + 1) * HB],
                                 start=(d == 0), stop=(d == ND - 1))
            if (tb, m) not in raw_sb:
                raw_sb[(tb, m)] = work.tile([128, TB], BF16, tag=f"raw{m}",
                                            name=f"raw{m}_{tb}")
            nc.vector.tensor_copy(raw_sb[(tb, m)][:, c * HB:(c + 1) * HB], ps)

        def emit_v_half(tb, s):
            ts = tb * 4 + s
            h = next_half()
            ps = mm2[:, h, :]
            for d in range(ND):
                nc.tensor.matmul(ps, lhsT=x_chunk(tb, d)[:, s * 128:(s + 1) * 128],
                                 rhs=wv_sb[:, d, :],
                                 start=(d == 0), stop=(d == ND - 1))
            nc.vector.tensor_copy(v_sb[ts], ps)

        def emit_rope(tb, m):
            """DVE-only: qkr = raw*cos + shuffle(raw)*sin_signed."""
            r = raw_sb.pop((tb, m))
            rot = work.tile([128, TB], BF16, tag="rot")
            nc.vector.stream_shuffle(rot, r, SHUF)
            qc = work.tile([128, TB], BF16, tag="qc")
            nc.vector.tensor_mul(qc, r, cos_sb[:, tb * TB:(tb + 1) * TB])
            rs = work.tile([128, TB], BF16, tag="rs")
            nc.vector.tensor_mul(rs, rot, sin_sb[:, tb * TB:(tb + 1) * TB])
            nc.vector.tensor_add(qk_rope[m][tb], qc, rs)

        def emit_proj_half(qb, s, n, c):
            h = next_half()
            ps = mm2[:, h, :]
            for p in range(2):
                nc.tensor.matmul(
                    ps, lhsT=outT_sb[(qb, p)][:, s * 128:(s + 1) * 128],
                    rhs=wpr_sb[:, p, n * TB + c * HB:n * TB + (c + 1) * HB],
                    start=(p == 0), stop=(p == 1))
            if (qb, s) not in pr_tiles:
                pr_tiles[(qb, s)] = prp.tile([128, 2, TB], BF16, tag="pr",
                                             name=f"pr{qb}_{s}")
            prt = pr_tiles[(qb, s)]
            nc.vector.tensor_copy(prt[:, n, c * HB:(c + 1) * HB], ps)
            if n == 1 and c == 1:
                nc.gpsimd.dma_start(
                    out=out_d[:, (qb * 4 + s) * DIM:(qb * 4 + s + 1) * DIM],
                    in_=prt.rearrange("p a b -> p (a b)"))

        # ---------- attention pieces ----------
        av_ps = {}
        rsum_ps = {}

        def emit_scores_exp(qb, kt):
            ktl = kt - 4 * qb
            a = 128 * ktl if ktl >= 0 else 0
            w = TB - a
            tbk, ok = kt // 4, (kt % 4) * 128
            for p in range(2):
                sc2 = ps_sc.tile([128, 2, TB], F32, tag="sc", name=f"sc{qb}_{kt}_{p}")
                for j in range(2):
                    nc.tensor.matmul(
                        sc2[:, j, 0:w],
                        lhsT=qk_rope[2 + p][tbk][64 * j:64 * j + 64, ok:ok + 128],
                        rhs=qk_rope[p][qb][64 * j:64 * j + 64, a:TB],
                        start=True, stop=True, tile_position=(64 * j, 0),
                    )
                exp2 = expp.tile([128, 2, TB], BF16, tag="exp", name=f"exp{qb}_{kt}_{p}")
                nc.scalar.activation(exp2[:, :, 0:w], sc2[:, :, 0:w],
                                     mybir.ActivationFunctionType.Exp,
                                     scale=float(SCALE))
                if ktl >= 0:
                    nc.vector.tensor_mul(exp2[:, :, 0:128], exp2[:, :, 0:128],
                                         mask_bc)
                expd[(qb, kt, p)] = exp2

        expd = {}

        def emit_av_rsum(qb, kt):
            nkt = 4 * (qb + 1)
            ktl = kt - 4 * qb
            a = 128 * ktl if ktl >= 0 else 0
            w = TB - a
            for p in range(2):
                exp2 = expd.pop((qb, kt, p))
                for j in range(2):
                    h = 2 * p + j
                    nc.tensor.matmul(
                        av_ps[(qb, p)][64 * j:64 * j + 64, a:TB],
                        lhsT=v_sb[kt][:, 64 * h:64 * h + 64],
                        rhs=exp2[:, j, 0:w],
                        start=(kt == 0), stop=(kt == nkt - 1),
                        skip_group_check=True,
                        tile_position=(0, 64 * j),
                    )
                for j in range(2):
                    h = 2 * p + j
                    nc.tensor.matmul(
                        rsum_ps[qb][32 * h:32 * h + 1, a:TB],
                        lhsT=ones_sb,
                        rhs=exp2[:, j, 0:w],
                        start=(kt == 0), stop=(kt == nkt - 1),
                        skip_group_check=True,
                        tile_position=(0, 32 * h),
                    )

        def emit_norm(qb):
            rsum_sb = work.tile([128, TB], BF16, tag="recip")
            nc.vector.tensor_copy(rsum_sb, rsum_ps[qb])
            bc2 = ps_sc.tile([128, 2, TB], F32, tag="sc", name=f"bc{qb}")
            for p in range(2):
                for j in range(2):
                    h = 2 * p + j
                    nc.tensor.matmul(
                        bc2[64 * j:64 * j + 64, p, :],
                        lhsT=ones2_sb[32 * h:32 * h + 1, :],
                        rhs=rsum_sb[32 * h:32 * h + 1, :],
                        start=True, stop=True, skip_group_check=True,
                        tile_position=(32 * h, 64 * j),
                    )
            recip2_sb = work.tile([128, 2, TB], F32, tag="recipb")
            nc.vector.reciprocal_approx_fast(out=recip2_sb, in_=bc2)
            for p in range(2):
                o_t = persist.tile([128, TB], BF16, tag=f"outT{qb}_{p}",
                                   name=f"outT{qb}_{p}")
                nc.vector.tensor_mul(o_t, av_ps[(qb, p)], recip2_sb[:, p, :])
                outT_sb[(qb, p)] = o_t

        # ---------- pre-loop: QKV q/k for block 0 + rope(0) ----------
        for m in range(4):
            ps = ps_acc.tile([128, TB], F32, tag=["av0", "av1", "rsum"][m % 3],
                             name=f"qk1_{m}_0")
            for d in range(ND):
                nc.tensor.matmul(ps, lhsT=wqk_chunk(d, m), rhs=x_chunk(0, d),
                                 start=(d == 0), stop=(d == ND - 1))
            r = work.tile([128, TB], BF16, tag=f"raw{m}", name=f"raw{m}_0")
            nc.vector.tensor_copy(r, ps)
            raw_sb[(0, m)] = r
        for m in (0, 2, 1, 3):
            emit_rope(0, m)

        # ---------- main loop: one-behind pipelined attention ----------
        pending = None           # (qb, kt) with exp done, av/rsum not yet emitted
        norm_due = None          # qb whose norm should be emitted after av flush
        for tb in range(NTB):
            qb = tb
            nkt = 4 * (qb + 1)
            if tb + 2 < NTB:
                xn = xt_pool.tile([128, ND, TB], BF16, tag="x", name=f"x_{tb + 2}")
                nc.sync.dma_start(
                    out=xn.rearrange("p a b -> p (a b)"),
                    in_=x_til[:, (tb + 2) * ND * TB:(tb + 3) * ND * TB])
                xt_sb[tb + 2] = xn

            units = []
            if tb == 0:
                units += [lambda s=s: emit_v_half(0, s) for s in range(4)]
            if tb + 1 < NTB:
                for m in range(4):
                    units += [lambda m=m, c=c: emit_qk_half(tb + 1, m, c)
                              for c in range(2)]
                    units += [lambda m=m: emit_rope(tb + 1, m)]
                units += [lambda s=s: emit_v_half(tb + 1, s) for s in range(4)]
            if tb - 1 >= 0:
                units += [lambda s=s, n=n, c=c: emit_proj_half(tb - 1, s, n, c)
                          for s in range(4) for n in range(2) for c in range(2)]
            nu = len(units)
            ui = 0

            av_ps[(qb, 0)] = ps_acc.tile([128, TB], F32, tag="av0", name=f"av0_{qb}")
            av_ps[(qb, 1)] = ps_acc.tile([128, TB], F32, tag="av1", name=f"av1_{qb}")
            rsum_ps[qb] = ps_acc.tile([128, TB], F32, tag="rsum", name=f"rsum_{qb}")

            for kt in range(nkt):
                emit_scores_exp(qb, kt)
                if pending is not None:
                    emit_av_rsum(*pending)
                pending = (qb, kt)
                if norm_due is not None:
                    emit_norm(norm_due)
                    norm_due = None
                want = (kt + 1) * nu // nkt
                while ui < want:
                    units[ui]()
                    ui += 1
            # flush this block's last av/rsum, then norm (emitted inside the
            # next block's first kt iteration to keep the PE queue busy)
            emit_av_rsum(*pending)
            pending = None
            norm_due = qb

        emit_norm(norm_due)

        # ---------- tail: final block's output projection ----------
        for s in range(4):
            for n in range(2):
                for c in range(2):
                    emit_proj_half(NTB - 1, s, n, c)


def shard_inputs(x, w_qkv, w_proj):
    """Full inputs -> list of 8 per-core input maps (pre-tiled layouts)."""
    cosT, sinT2, mask01 = _host_constants()
    x = np.ascontiguousarray(np.asarray(x, dtype=np.float32))
    w_qkv = np.asarray(w_qkv, dtype=np.float32)
    w_proj = np.asarray(w_proj, dtype=np.float32)
    bf = ml_dtypes.bfloat16
    in_maps = []
    for c in range(N_CORES):
        b, g = c // TPG, c % TPG
        xT = x[b].T                                           # [DIM, T]
        xt4 = xT.reshape(ND, 128, NTB, TB)                    # [d, p, tb, j]
        x_til = np.ascontiguousarray(xt4.transpose(1, 2, 0, 3).reshape(128, NTB * ND * TB))
        # permute each head's 64 q/k dims by PERM64 (see SHUF comment)
        pidx = (np.arange(LOC) // HEAD_DIM) * HEAD_DIM + np.array(
            [PERM64[i % HEAD_DIM] for i in range(LOC)])
        wq = w_qkv[:, g * LOC:(g + 1) * LOC][:, pidx]
        wk = w_qkv[:, INNER + g * LOC:INNER + (g + 1) * LOC][:, pidx]
        wv = w_qkv[:, 2 * INNER + g * LOC:2 * INNER + (g + 1) * LOC]
        w_qk = np.concatenate([wq, wk], axis=1)               # [DIM, 512]
        wqk_t = w_qk.reshape(ND, 128, 2 * LOC)                # [d, p, c]
        wqk0 = np.ascontiguousarray(wqk_t[0])
        wqkR = np.ascontiguousarray(wqk_t[1:].transpose(1, 0, 2).reshape(128, (ND - 1) * 2 * LOC))
        wv_t = np.ascontiguousarray(
            wv.reshape(ND, 128, LOC).transpose(1, 0, 2).reshape(128, ND * LOC))
        w_pr = w_proj[g * LOC:(g + 1) * LOC, :]               # [256, DIM]
        wpr_t = np.ascontiguousarray(
            w_pr.reshape(2, 128, DIM).transpose(1, 0, 2).reshape(128, 2 * DIM))
        in_maps.append({
            "x_til": x_til.astype(bf),
            "wqk0": wqk0.astype(bf),
            "wqkR": wqkR.astype(bf),
            "wv_til": wv_t.astype(bf),
            "wpr_til": wpr_t.astype(bf),
            "cosT": cosT.astype(bf),
            "sinT2": sinT2.astype(bf),
            "mask01": mask01.astype(bf),
        })
    return in_maps


_CACHE = {}


def _get_compiled():
    if "nc" not in _CACHE:
        nc = bacc.Bacc("TRN2", target_bir_lowering=False, debug=False,
                       enable_asserts=True, num_devices=N_CORES)
        with tile.TileContext(nc) as tc:
            build_kernel(tc)
        nc.compile()
        _CACHE["nc"] = nc
    return _CACHE["nc"]


def kernel(x, w_qkv, w_proj):
    nc = _get_compiled()
    in_maps = shard_inputs(x, w_qkv, w_proj)
    res = run_bass_kernel_spmd(nc, in_maps, core_ids=list(range(N_CORES)))
    outs = []
    for c in range(N_CORES):
        o = np.asarray(res.results[c]["out"], dtype=np.float32)
        o = o.reshape(128, 16, DIM).transpose(1, 0, 2).reshape(T, DIM)
        outs.append(o)
    full = np.stack([
        np.sum([outs[b * TPG + g] for g in range(TPG)], axis=0, dtype=np.float32)
        for b in range(B)
    ])
    return full.astype(np.float32)


# revision 20
# speedup vs baseline: 1.1433x; 1.0640x over previous
"""Causal self-attention with RoPE on 8 TRN2 NeuronCores.

Sharding: 2 (batch) x 4 (head-group tensor parallel). Core c handles
batch b=c//4 and heads [4g, 4g+4) with g=c%4. Each core computes its
q,k,v projections, RoPE, causal attention (transposed-scores flash
layout), and its partial of the output projection; the host sums the
4 partials per batch (the "all-reduce").

v4: one-stage-pipelined attention (av/rowsum trail scores/exp by one
kt so the in-order PE queue never blocks on the exp semaphore); RoPE
rotation via DVE stream_shuffle with the sign folded into the sin
table (no PE rotation matmuls, no rope PSUM); QKV(tb+1)/proj(tb-1)
emitted as half-PSUM-bank ping-pong units inside attention(tb)'s kt
loop; startup DMAs split per 128-row chunk and issued from four
engine queues in parallel.

Self-contained: hardcodes shapes from the problem spec.
"""
import numpy as np
import ml_dtypes

import concourse.bass as bass
import concourse.mybir as mybir
import concourse.tile as tile
from concourse import bacc
from concourse.bass_utils import run_bass_kernel_spmd

F32 = mybir.dt.float32
BF16 = mybir.dt.bfloat16

B, T, DIM = 2, 2048, 1024
HEADS, HEAD_DIM = 16, 64
INNER = HEADS * HEAD_DIM
ROPE_BASE = 10000.0
N_CORES = 8
TPG = 4                      # tensor-parallel group size (head groups)
HPC = HEADS // TPG           # heads per core = 4
LOC = HPC * HEAD_DIM         # local inner = 256
SCALE = 1.0 / np.sqrt(HEAD_DIM)

TB = 512                     # t block for QKV / q block for attention
NTB = T // TB                # 4
ND = DIM // 128              # 8 contraction chunks
HB = 256                     # half-bank column count for ping-pong units

# stream_shuffle permutes within 32-partition blocks (mask replicated
# across blocks). We reorder each head's 64 q/k dims as
# [0..15, 32..47, 16..31, 48..63] so the rope partner (d <-> d+32) sits
# 16 partitions away inside the same 32-block; the shuffle is then a
# 16-half swap. Scores are invariant to this (same perm on q and k).
SHUF = list(range(16, 32)) + list(range(16))
PERM64 = list(range(16)) + list(range(32, 48)) + list(range(16, 32)) + list(range(48, 64))


def _host_constants():
    inv_freq = 1.0 / (ROPE_BASE ** (np.arange(0, HEAD_DIM, 2, dtype=np.float32) / HEAD_DIM))
    t = np.arange(T, dtype=np.float32)
    freqs = np.outer(t, inv_freq).astype(np.float32)          # [T, 32]
    cos32 = np.cos(freqs).T.astype(np.float32)                # [32, T]
    sin32 = np.sin(freqs).T.astype(np.float32)
    cos64 = np.tile(cos32, (2, 1))                            # [64, T]
    sin64 = np.tile(sin32, (2, 1))
    perm = np.array(PERM64)
    # per-head permuted tables; rotate-half sign folded into sin
    cosP = cos64[perm]                                        # [64, T]
    sgn = np.where(perm < 32, -1.0, 1.0)[:, None]
    sinP = sin64[perm] * sgn
    cosT = np.tile(cosP, (2, 1))                              # [128, T]
    sinT2 = np.tile(sinP, (2, 1))

    # post-exp 0/1 causal mask for the diagonal 128-col block: keep j >= p
    j = np.arange(128)[None, :]
    p = np.arange(128)[:, None]
    mask01 = (j >= p).astype(np.float32)                      # [128, 128]
    return cosT, sinT2, mask01


def build_kernel(tc):
    nc = tc.nc
    x_til = nc.dram_tensor("x_til", [128, NTB * ND * TB], BF16, kind="ExternalInput").ap()
    wqk0_d = nc.dram_tensor("wqk0", [128, 2 * LOC], BF16, kind="ExternalInput").ap()
    wqkR_d = nc.dram_tensor("wqkR", [128, (ND - 1) * 2 * LOC], BF16, kind="ExternalInput").ap()
    wv_d = nc.dram_tensor("wv_til", [128, ND * LOC], BF16, kind="ExternalInput").ap()
    wpr_d = nc.dram_tensor("wpr_til", [128, 2 * DIM], BF16, kind="ExternalInput").ap()
    cosT_d = nc.dram_tensor("cosT", [128, T], BF16, kind="ExternalInput").ap()
    sinT_d = nc.dram_tensor("sinT2", [128, T], BF16, kind="ExternalInput").ap()
    mask_d = nc.dram_tensor("mask01", [128, 128], BF16, kind="ExternalInput").ap()
    out_d = nc.dram_tensor("out", [128, NTB * 4 * DIM], BF16, kind="ExternalOutput").ap()

    with (
        tc.tile_pool(name="const", bufs=1) as const,
        tc.tile_pool(name="xt", bufs=2) as xt_pool,
        tc.tile_pool(name="persist", bufs=1) as persist,
        tc.tile_pool(name="work", bufs=4) as work,
        tc.tile_pool(name="prp", bufs=2) as prp,
        tc.tile_pool(name="expp", bufs=6) as expp,
        tc.tile_pool(name="ps_sc", bufs=2, space="PSUM") as ps_sc,
        tc.tile_pool(name="ps_acc", bufs=1, space="PSUM") as ps_acc,
        tc.tile_pool(name="ps_mm", bufs=1, space="PSUM") as ps_mm,
    ):
        # ---- startup DMAs in need-order: the pre-loop's x(0)/w_qk first
        # (split in chunks across the sync + scalar queues so the d-chains
        # can start as chunks land), everything else queued behind ----
        xt0a = const.tile([128, TB], BF16, tag="xt0a")
        nc.sync.dma_start(out=xt0a, in_=x_til[:, 0:TB])
        wqk0 = const.tile([128, 2 * LOC], BF16, tag="wqk0")
        nc.scalar.dma_start(out=wqk0, in_=wqk0_d)
        wqkR = const.tile([128, (ND - 1) * 2 * LOC], BF16, tag="wqkR")
        xt0b = const.tile([128, (ND - 1) * TB], BF16, tag="xt0b")
        for lo, hi in ((1, 4), (4, 8)):
            nc.sync.dma_start(out=xt0b[:, (lo - 1) * TB:(hi - 1) * TB],
                              in_=x_til[:, lo * TB:hi * TB])
            nc.scalar.dma_start(out=wqkR[:, (lo - 1) * 2 * LOC:(hi - 1) * 2 * LOC],
                                in_=wqkR_d[:, (lo - 1) * 2 * LOC:(hi - 1) * 2 * LOC])

        cos_sb = const.tile([128, T], BF16, tag="cos")
        nc.sync.dma_start(out=cos_sb, in_=cosT_d)
        sin_sb = const.tile([128, T], BF16, tag="sin")
        nc.scalar.dma_start(out=sin_sb, in_=sinT_d)
        wv_sb = const.tile([128, ND, LOC], BF16, tag="wv")
        nc.sync.dma_start(out=wv_sb.rearrange("p a b -> p (a b)"), in_=wv_d)
        mask_sb = const.tile([128, 128], BF16, tag="mask")
        nc.scalar.dma_start(out=mask_sb, in_=mask_d)

        ones_sb = const.tile([128, 1], BF16, tag="ones")
        nc.vector.memset(ones_sb, 1.0)
        ones2_sb = const.tile([128, 64], BF16, tag="ones2")
        nc.vector.memset(ones2_sb, 1.0)

        xt_sb = {}
        x1 = xt_pool.tile([128, ND, TB], BF16, tag="x", name="x_1")
        nc.sync.dma_start(out=x1.rearrange("p a b -> p (a b)"),
                          in_=x_til[:, ND * TB:2 * ND * TB])
        xt_sb[1] = x1
        wpr_sb = const.tile([128, 2, DIM], BF16, tag="wpr")
        nc.scalar.dma_start(out=wpr_sb.rearrange("p a b -> p (a b)"), in_=wpr_d)

        mask_bc = mask_sb.rearrange("p (o n) -> p o n", o=1).to_broadcast([128, 2, 128])

        def x_chunk(tb, d):
            if tb == 0:
                return xt0a if d == 0 else xt0b[:, (d - 1) * TB:d * TB]
            return xt_sb[tb][:, d, :]

        def wqk_chunk(d, m):
            if d == 0:
                return wqk0[:, m * 128:(m + 1) * 128]
            return wqkR[:, (d - 1) * 2 * LOC + m * 128:(d - 1) * 2 * LOC + (m + 1) * 128]

        # persistent per-phase outputs
        qk_rope = [[persist.tile([128, TB], BF16, tag=f"qkr{m}_{tb}", name=f"qkr{m}_{tb}")
                    for tb in range(NTB)] for m in range(4)]
        v_sb = [persist.tile([128, LOC], BF16, tag=f"v{ts}", name=f"v{ts}")
                for ts in range(4 * NTB)]
        raw_sb = {}
        outT_sb = {}
        pr_tiles = {}

        # the single ping-pong PSUM bank for pipelined QKV/v/proj units
        mm2 = ps_mm.tile([128, 2, HB], F32, tag="mm", name="mm2")
        half = [0]

        def next_half():
            h = half[0]
            half[0] ^= 1
            return h

        # ---------- emission units ----------
        def emit_qk_half(tb, m, c):
            """c in {0,1}: column half of the [128, TB] q/k pair output."""
            h = next_half()
            ps = mm2[:, h, :]
            for d in range(ND):
                nc.tensor.matmul(ps, lhsT=wqk_chunk(d, m),
                                 rhs=x_chunk(tb, d)[:, c * HB:(c + 1) * HB],
                                 start=(d == 0), stop=(d == ND - 1))
            if (tb, m) not in raw_sb:
                raw_sb[(tb, m)] = work.tile([128, TB], BF16, tag=f"raw{m}",
                                            name=f"raw{m}_{tb}")
            nc.vector.tensor_copy(raw_sb[(tb, m)][:, c * HB:(c + 1) * HB], ps)

        def emit_v_half(tb, s):
            ts = tb * 4 + s
            h = next_half()
            ps = mm2[:, h, :]
            for d in range(ND):
                nc.tensor.matmul(ps, lhsT=x_chunk(tb, d)[:, s * 128:(s + 1) * 128],
                                 rhs=wv_sb[:, d, :],
                                 start=(d == 0), stop=(d == ND - 1))
            nc.vector.tensor_copy(v_sb[ts], ps)

        def emit_rope(tb, m):
            """DVE-only: qkr = raw*cos + shuffle(raw)*sin_signed."""
            r = raw_sb.pop((tb, m))
            rot = work.tile([128, TB], BF16, tag="rot")
            nc.vector.stream_shuffle(rot, r, SHUF)
            qc = work.tile([128, TB], BF16, tag="qc")
            nc.vector.tensor_mul(qc, r, cos_sb[:, tb * TB:(tb + 1) * TB])
            rs = work.tile([128, TB], BF16, tag="rs")
            nc.vector.tensor_mul(rs, rot, sin_sb[:, tb * TB:(tb + 1) * TB])
            nc.vector.tensor_add(qk_rope[m][tb], qc, rs)

        def emit_proj_half(qb, s, n, c):
            h = next_half()
            ps = mm2[:, h, :]
            for p in range(2):
                nc.tensor.matmul(
                    ps, lhsT=outT_sb[(qb, p)][:, s * 128:(s + 1) * 128],
                    rhs=wpr_sb[:, p, n * TB + c * HB:n * TB + (c + 1) * HB],
                    start=(p == 0), stop=(p == 1))
            if (qb, s) not in pr_tiles:
                pr_tiles[(qb, s)] = prp.tile([128, 2, TB], BF16, tag="pr",
                                             name=f"pr{qb}_{s}")
            prt = pr_tiles[(qb, s)]
            nc.vector.tensor_copy(prt[:, n, c * HB:(c + 1) * HB], ps)
            if n == 1 and c == 1:
                nc.gpsimd.dma_start(
                    out=out_d[:, (qb * 4 + s) * DIM:(qb * 4 + s + 1) * DIM],
                    in_=prt.rearrange("p a b -> p (a b)"))

        # ---------- attention pieces ----------
        av_ps = {}
        rsum_ps = {}

        expd = {}

        def emit_scores_exp(ph, kt):
            qb, W0, QW, nkt, db = ph
            ktl = kt - db
            a = 128 * ktl if ktl >= 0 else 0
            w = QW - a
            tbk, ok = kt // 4, (kt % 4) * 128
            for p in range(2):
                sc2 = ps_sc.tile([128, 2, TB], F32, tag="sc",
                                 name=f"sc{qb}_{W0}_{kt}_{p}")
                for j in range(2):
                    nc.tensor.matmul(
                        sc2[:, j, 0:w],
                        lhsT=qk_rope[2 + p][tbk][64 * j:64 * j + 64, ok:ok + 128],
                        rhs=qk_rope[p][qb][64 * j:64 * j + 64, W0 + a:W0 + QW],
                        start=True, stop=True, tile_position=(64 * j, 0),
                    )
                exp2 = expp.tile([128, 2, TB], BF16, tag="exp",
                                 name=f"exp{qb}_{W0}_{kt}_{p}")
                nc.scalar.activation(exp2[:, :, 0:w], sc2[:, :, 0:w],
                                     mybir.ActivationFunctionType.Exp,
                                     scale=float(SCALE))
                if ktl >= 0:
                    nc.vector.tensor_mul(exp2[:, :, 0:128], exp2[:, :, 0:128],
                                         mask_bc)
                expd[(qb, W0, kt, p)] = exp2

        def emit_av_rsum(ph, kt):
            qb, W0, QW, nkt, db = ph
            ktl = kt - db
            a = 128 * ktl if ktl >= 0 else 0
            w = QW - a
            for p in range(2):
                exp2 = expd.pop((qb, W0, kt, p))
                for j in range(2):
                    h = 2 * p + j
                    nc.tensor.matmul(
                        av_ps[(qb, W0, p)][64 * j:64 * j + 64, a:QW],
                        lhsT=v_sb[kt][:, 64 * h:64 * h + 64],
                        rhs=exp2[:, j, 0:w],
                        start=(kt == 0), stop=(kt == nkt - 1),
                        skip_group_check=True,
                        tile_position=(0, 64 * j),
                    )
                for j in range(2):
                    h = 2 * p + j
                    nc.tensor.matmul(
                        rsum_ps[(qb, W0)][32 * h:32 * h + 1, a:QW],
                        lhsT=ones_sb,
                        rhs=exp2[:, j, 0:w],
                        start=(kt == 0), stop=(kt == nkt - 1),
                        skip_group_check=True,
                        tile_position=(0, 32 * h),
                    )

        def emit_norm(ph):
            qb, W0, QW, nkt, db = ph
            rsum_sb = work.tile([128, TB], BF16, tag="recip")
            nc.vector.tensor_copy(rsum_sb[:, 0:QW], rsum_ps[(qb, W0)][:, 0:QW])
            bc2 = ps_sc.tile([128, 2, TB], F32, tag="sc", name=f"bc{qb}_{W0}")
            for p in range(2):
                for j in range(2):
                    h = 2 * p + j
                    nc.tensor.matmul(
                        bc2[64 * j:64 * j + 64, p, 0:QW],
                        lhsT=ones2_sb[32 * h:32 * h + 1, :],
                        rhs=rsum_sb[32 * h:32 * h + 1, 0:QW],
                        start=True, stop=True, skip_group_check=True,
                        tile_position=(32 * h, 64 * j),
                    )
            recip2_sb = work.tile([128, 2, TB], F32, tag="recipb")
            nc.vector.reciprocal_approx_fast(out=recip2_sb[:, :, 0:QW],
                                             in_=bc2[:, :, 0:QW])
            for p in range(2):
                if (qb, p) not in outT_sb:
                    outT_sb[(qb, p)] = persist.tile(
                        [128, TB], BF16, tag=f"outT{qb}_{p}", name=f"outT{qb}_{p}")
                nc.vector.tensor_mul(outT_sb[(qb, p)][:, W0:W0 + QW],
                                     av_ps[(qb, W0, p)][:, 0:QW],
                                     recip2_sb[:, p, 0:QW])

        # ---------- pre-loop: QKV q/k for block 0 + rope(0) ----------
        for m in range(4):
            ps = ps_acc.tile([128, TB], F32, tag=["av0", "av1", "rsum"][m % 3],
                             name=f"qk1_{m}_0")
            for d in range(ND):
                nc.tensor.matmul(ps, lhsT=wqk_chunk(d, m), rhs=x_chunk(0, d),
                                 start=(d == 0), stop=(d == ND - 1))
            r = work.tile([128, TB], BF16, tag=f"raw{m}", name=f"raw{m}_0")
            nc.vector.tensor_copy(r, ps)
            raw_sb[(0, m)] = r
        for m in (0, 2, 1, 3):
            emit_rope(0, m)

        # ---------- main loop: one-behind pipelined attention ----------
        # phase = (qb, W0, QW, nkt, diag_base); the last block is split into
        # two q-column halves so its norm+proj overlap the second half
        phases = [(tb, 0, TB, 4 * (tb + 1), 4 * tb) for tb in range(NTB)]

        def phase_units(qb, W0):
            units = []
            if qb == 0:
                units += [lambda s=s: emit_v_half(0, s) for s in range(4)]
            if W0 == 0 and qb + 1 < NTB:
                for m in range(4):
                    units += [lambda m=m, c=c: emit_qk_half(qb + 1, m, c)
                              for c in range(2)]
                    units += [lambda m=m: emit_rope(qb + 1, m)]
                units += [lambda s=s: emit_v_half(qb + 1, s) for s in range(4)]
            if W0 == 0 and qb - 1 >= 0:
                units += [lambda s=s, n=n, c=c: emit_proj_half(qb - 1, s, n, c)
                          for s in range(4) for n in range(2) for c in range(2)]
            return units

        pending = None           # (phase, kt) with exp done, av/rsum pending
        norm_due = None
        for ph in phases:
            qb, W0, QW, nkt, db = ph
            if W0 == 0 and qb + 2 < NTB:
                xn = xt_pool.tile([128, ND, TB], BF16, tag="x", name=f"x_{qb + 2}")
                nc.sync.dma_start(
                    out=xn.rearrange("p a b -> p (a b)"),
                    in_=x_til[:, (qb + 2) * ND * TB:(qb + 3) * ND * TB])
                xt_sb[qb + 2] = xn

            units = phase_units(qb, W0)
            nu = len(units)
            ui = 0
            av_ps[(qb, W0, 0)] = ps_acc.tile([128, TB], F32, tag="av0",
                                             name=f"av0_{qb}_{W0}")
            av_ps[(qb, W0, 1)] = ps_acc.tile([128, TB], F32, tag="av1",
                                             name=f"av1_{qb}_{W0}")
            rsum_ps[(qb, W0)] = ps_acc.tile([128, TB], F32, tag="rsum",
                                            name=f"rsum_{qb}_{W0}")

            for kt in range(nkt):
                emit_scores_exp(ph, kt)
                if pending is not None:
                    emit_av_rsum(*pending)
                pending = (ph, kt)
                if norm_due is not None:
                    emit_norm(norm_due)
                    norm_due = None
                # hold units back one kt so a fresh phase's norm lands
                # first; front-load on the last (ACT-bound) phase
                if qb == NTB - 1:
                    want = 0 if kt == 0 else min(nu, (kt + 1) * nu * 2 // nkt)
                else:
                    want = kt * nu // max(nkt - 1, 1)
                while ui < want:
                    units[ui]()
                    ui += 1
            while ui < nu:
                units[ui]()
                ui += 1
            emit_av_rsum(*pending)
            pending = None
            norm_due = ph

        emit_norm(norm_due)

        # ---------- tail: final half-block's output projection ----------
        for s in (0, 1, 2, 3):
            for n in range(2):
                for c in range(2):
                    emit_proj_half(NTB - 1, s, n, c)


def shard_inputs(x, w_qkv, w_proj):
    """Full inputs -> list of 8 per-core input maps (pre-tiled layouts)."""
    cosT, sinT2, mask01 = _host_constants()
    x = np.ascontiguousarray(np.asarray(x, dtype=np.float32))
    w_qkv = np.asarray(w_qkv, dtype=np.float32)
    w_proj = np.asarray(w_proj, dtype=np.float32)
    bf = ml_dtypes.bfloat16
    in_maps = []
    for c in range(N_CORES):
        b, g = c // TPG, c % TPG
        xT = x[b].T                                           # [DIM, T]
        xt4 = xT.reshape(ND, 128, NTB, TB)                    # [d, p, tb, j]
        x_til = np.ascontiguousarray(xt4.transpose(1, 2, 0, 3).reshape(128, NTB * ND * TB))
        # permute each head's 64 q/k dims by PERM64 (see SHUF comment)
        pidx = (np.arange(LOC) // HEAD_DIM) * HEAD_DIM + np.array(
            [PERM64[i % HEAD_DIM] for i in range(LOC)])
        wq = w_qkv[:, g * LOC:(g + 1) * LOC][:, pidx]
        wk = w_qkv[:, INNER + g * LOC:INNER + (g + 1) * LOC][:, pidx]
        wv = w_qkv[:, 2 * INNER + g * LOC:2 * INNER + (g + 1) * LOC]
        w_qk = np.concatenate([wq, wk], axis=1)               # [DIM, 512]
        wqk_t = w_qk.reshape(ND, 128, 2 * LOC)                # [d, p, c]
        wqk0 = np.ascontiguousarray(wqk_t[0])
        wqkR = np.ascontiguousarray(wqk_t[1:].transpose(1, 0, 2).reshape(128, (ND - 1) * 2 * LOC))
        wv_t = np.ascontiguousarray(
            wv.reshape(ND, 128, LOC).transpose(1, 0, 2).reshape(128, ND * LOC))
        w_pr = w_proj[g * LOC:(g + 1) * LOC, :]               # [256, DIM]
        wpr_t = np.ascontiguousarray(
            w_pr.reshape(2, 128, DIM).transpose(1, 0, 2).reshape(128, 2 * DIM))
        in_maps.append({
            "x_til": x_til.astype(bf),
            "wqk0": wqk0.astype(bf),
            "wqkR": wqkR.astype(bf),
            "wv_til": wv_t.astype(bf),
            "wpr_til": wpr_t.astype(bf),
            "cosT": cosT.astype(bf),
            "sinT2": sinT2.astype(bf),
            "mask01": mask01.astype(bf),
        })
    return in_maps


_CACHE = {}


def _get_compiled():
    if "nc" not in _CACHE:
        nc = bacc.Bacc("TRN2", target_bir_lowering=False, debug=False,
                       enable_asserts=True, num_devices=N_CORES)
        with tile.TileContext(nc) as tc:
            build_kernel(tc)
        nc.compile()
        _CACHE["nc"] = nc
    return _CACHE["nc"]


def kernel(x, w_qkv, w_proj):
    nc = _get_compiled()
    in_maps = shard_inputs(x, w_qkv, w_proj)
    res = run_bass_kernel_spmd(nc, in_maps, core_ids=list(range(N_CORES)))
    outs = []
    for c in range(N_CORES):
        o = np.asarray(res.results[c]["out"], dtype=np.float32)
        o = o.reshape(128, 16, DIM).transpose(1, 0, 2).reshape(T, DIM)
        outs.append(o)
    full = np.stack([
        np.sum([outs[b * TPG + g] for g in range(TPG)], axis=0, dtype=np.float32)
        for b in range(B)
    ])
    return full.astype(np.float32)


# revision 23
# speedup vs baseline: 1.1857x; 1.0370x over previous
"""Causal self-attention with RoPE on 8 TRN2 NeuronCores.

Sharding: 2 (batch) x 4 (head-group tensor parallel). Core c handles
batch b=c//4 and heads [4g, 4g+4) with g=c%4. Each core computes its
q,k,v projections, RoPE, causal attention (transposed-scores flash
layout), and its partial of the output projection; the host sums the
4 partials per batch (the "all-reduce").

v4: one-stage-pipelined attention (av/rowsum trail scores/exp by one
kt so the in-order PE queue never blocks on the exp semaphore); RoPE
rotation via DVE stream_shuffle with the sign folded into the sin
table (no PE rotation matmuls, no rope PSUM); QKV(tb+1)/proj(tb-1)
emitted as half-PSUM-bank ping-pong units inside attention(tb)'s kt
loop; startup DMAs split per 128-row chunk and issued from four
engine queues in parallel.

Self-contained: hardcodes shapes from the problem spec.
"""
import numpy as np
import ml_dtypes

import concourse.bass as bass
import concourse.mybir as mybir
import concourse.tile as tile
from concourse import bacc
from concourse.bass_utils import run_bass_kernel_spmd

F32 = mybir.dt.float32
BF16 = mybir.dt.bfloat16

B, T, DIM = 2, 2048, 1024
HEADS, HEAD_DIM = 16, 64
INNER = HEADS * HEAD_DIM
ROPE_BASE = 10000.0
N_CORES = 8
TPG = 4                      # tensor-parallel group size (head groups)
HPC = HEADS // TPG           # heads per core = 4
LOC = HPC * HEAD_DIM         # local inner = 256
SCALE = 1.0 / np.sqrt(HEAD_DIM)

TB = 512                     # t block for QKV / q block for attention
NTB = T // TB                # 4
ND = DIM // 128              # 8 contraction chunks
HB = 256                     # half-bank column count for ping-pong units

# stream_shuffle permutes within 32-partition blocks (mask replicated
# across blocks). We reorder each head's 64 q/k dims as
# [0..15, 32..47, 16..31, 48..63] so the rope partner (d <-> d+32) sits
# 16 partitions away inside the same 32-block; the shuffle is then a
# 16-half swap. Scores are invariant to this (same perm on q and k).
SHUF = list(range(16, 32)) + list(range(16))
PERM64 = list(range(16)) + list(range(32, 48)) + list(range(16, 32)) + list(range(48, 64))


def _host_constants():
    inv_freq = 1.0 / (ROPE_BASE ** (np.arange(0, HEAD_DIM, 2, dtype=np.float32) / HEAD_DIM))
    t = np.arange(T, dtype=np.float32)
    freqs = np.outer(t, inv_freq).astype(np.float32)          # [T, 32]
    cos32 = np.cos(freqs).T.astype(np.float32)                # [32, T]
    sin32 = np.sin(freqs).T.astype(np.float32)
    cos64 = np.tile(cos32, (2, 1))                            # [64, T]
    sin64 = np.tile(sin32, (2, 1))
    perm = np.array(PERM64)
    # per-head permuted tables; rotate-half sign folded into sin
    cosP = cos64[perm]                                        # [64, T]
    sgn = np.where(perm < 32, -1.0, 1.0)[:, None]
    sinP = sin64[perm] * sgn
    cosT = np.tile(cosP, (2, 1))                              # [128, T]
    sinT2 = np.tile(sinP, (2, 1))

    # post-exp 0/1 causal mask for the diagonal 128-col block: keep j >= p
    j = np.arange(128)[None, :]
    p = np.arange(128)[:, None]
    mask01 = (j >= p).astype(np.float32)                      # [128, 128]
    return cosT, sinT2, mask01


def build_kernel(tc):
    nc = tc.nc
    x_til = nc.dram_tensor("x_til", [128, NTB * ND * TB], BF16, kind="ExternalInput").ap()
    wqk0_d = nc.dram_tensor("wqk0", [128, 2 * LOC], BF16, kind="ExternalInput").ap()
    wqkR_d = nc.dram_tensor("wqkR", [128, (ND - 1) * 2 * LOC], BF16, kind="ExternalInput").ap()
    wv_d = nc.dram_tensor("wv_til", [128, ND * LOC], BF16, kind="ExternalInput").ap()
    wpr_d = nc.dram_tensor("wpr_til", [128, 2 * DIM], BF16, kind="ExternalInput").ap()
    cosT_d = nc.dram_tensor("cosT", [128, T], BF16, kind="ExternalInput").ap()
    sinT_d = nc.dram_tensor("sinT2", [128, T], BF16, kind="ExternalInput").ap()
    mask_d = nc.dram_tensor("mask01", [128, 128], BF16, kind="ExternalInput").ap()
    out_d = nc.dram_tensor("out", [128, NTB * 4 * DIM], BF16, kind="ExternalOutput").ap()

    with (
        tc.tile_pool(name="const", bufs=1) as const,
        tc.tile_pool(name="xt", bufs=2) as xt_pool,
        tc.tile_pool(name="persist", bufs=1) as persist,
        tc.tile_pool(name="work", bufs=4) as work,
        tc.tile_pool(name="prp", bufs=2) as prp,
        tc.tile_pool(name="expp", bufs=6) as expp,
        tc.tile_pool(name="ps_sc", bufs=2, space="PSUM") as ps_sc,
        tc.tile_pool(name="ps_acc", bufs=1, space="PSUM") as ps_acc,
        tc.tile_pool(name="ps_mm", bufs=1, space="PSUM") as ps_mm,
    ):
        # ---- startup DMAs in need-order: the pre-loop's x(0)/w_qk first
        # (split in chunks across the sync + scalar queues so the d-chains
        # can start as chunks land), everything else queued behind ----
        xt0a = const.tile([128, TB], BF16, tag="xt0a")
        nc.sync.dma_start(out=xt0a, in_=x_til[:, 0:TB])
        wqk0 = const.tile([128, 2 * LOC], BF16, tag="wqk0")
        nc.scalar.dma_start(out=wqk0, in_=wqk0_d)
        wqkR = const.tile([128, (ND - 1) * 2 * LOC], BF16, tag="wqkR")
        xt0b = const.tile([128, (ND - 1) * TB], BF16, tag="xt0b")
        for lo, hi in ((1, 4), (4, 8)):
            nc.sync.dma_start(out=xt0b[:, (lo - 1) * TB:(hi - 1) * TB],
                              in_=x_til[:, lo * TB:hi * TB])
            nc.scalar.dma_start(out=wqkR[:, (lo - 1) * 2 * LOC:(hi - 1) * 2 * LOC],
                                in_=wqkR_d[:, (lo - 1) * 2 * LOC:(hi - 1) * 2 * LOC])

        cos_sb = const.tile([128, T], BF16, tag="cos")
        nc.sync.dma_start(out=cos_sb, in_=cosT_d)
        sin_sb = const.tile([128, T], BF16, tag="sin")
        nc.scalar.dma_start(out=sin_sb, in_=sinT_d)
        wv_sb = const.tile([128, ND, LOC], BF16, tag="wv")
        nc.sync.dma_start(out=wv_sb.rearrange("p a b -> p (a b)"), in_=wv_d)
        mask_sb = const.tile([128, 128], BF16, tag="mask")
        nc.scalar.dma_start(out=mask_sb, in_=mask_d)

        ones_sb = const.tile([128, 1], BF16, tag="ones")
        nc.vector.memset(ones_sb, 1.0)
        ones2_sb = const.tile([128, 64], BF16, tag="ones2")
        nc.vector.memset(ones2_sb, 1.0)


        xt_sb = {}
        x1 = xt_pool.tile([128, ND, TB], BF16, tag="x", name="x_1")
        nc.sync.dma_start(out=x1.rearrange("p a b -> p (a b)"),
                          in_=x_til[:, ND * TB:2 * ND * TB])
        xt_sb[1] = x1
        wpr_sb = const.tile([128, 2, DIM], BF16, tag="wpr")
        nc.scalar.dma_start(out=wpr_sb.rearrange("p a b -> p (a b)"), in_=wpr_d)

        mask_bc = mask_sb.rearrange("p (o n) -> p o n", o=1).to_broadcast([128, 2, 128])

        def x_chunk(tb, d):
            if tb == 0:
                return xt0a if d == 0 else xt0b[:, (d - 1) * TB:d * TB]
            return xt_sb[tb][:, d, :]

        def wqk_chunk(d, m):
            if d == 0:
                return wqk0[:, m * 128:(m + 1) * 128]
            return wqkR[:, (d - 1) * 2 * LOC + m * 128:(d - 1) * 2 * LOC + (m + 1) * 128]

        # persistent per-phase outputs
        qk_rope = [[persist.tile([128, TB], BF16, tag=f"qkr{m}_{tb}", name=f"qkr{m}_{tb}")
                    for tb in range(NTB)] for m in range(4)]
        v_sb = [persist.tile([128, LOC], BF16, tag=f"v{ts}", name=f"v{ts}")
                for ts in range(4 * NTB)]
        raw_sb = {}
        outT_sb = {}
        pr_tiles = {}

        # the single ping-pong PSUM bank for pipelined QKV/v/proj units
        mm2 = ps_mm.tile([128, 2, HB], F32, tag="mm", name="mm2")
        half = [0]

        def next_half():
            h = half[0]
            half[0] ^= 1
            return h

        # ---------- emission units ----------
        def emit_qk_half(tb, m, c):
            """c in {0,1}: column half of the [128, TB] q/k pair output."""
            h = next_half()
            ps = mm2[:, h, :]
            for d in range(ND):
                nc.tensor.matmul(ps, lhsT=wqk_chunk(d, m),
                                 rhs=x_chunk(tb, d)[:, c * HB:(c + 1) * HB],
                                 start=(d == 0), stop=(d == ND - 1))
            if (tb, m) not in raw_sb:
                raw_sb[(tb, m)] = work.tile([128, TB], BF16, tag=f"raw{m}",
                                            name=f"raw{m}_{tb}")
            nc.vector.tensor_copy(raw_sb[(tb, m)][:, c * HB:(c + 1) * HB], ps)

        def emit_v_half(tb, s):
            ts = tb * 4 + s
            h = next_half()
            ps = mm2[:, h, :]
            for d in range(ND):
                nc.tensor.matmul(ps, lhsT=x_chunk(tb, d)[:, s * 128:(s + 1) * 128],
                                 rhs=wv_sb[:, d, :],
                                 start=(d == 0), stop=(d == ND - 1))
            nc.vector.tensor_copy(v_sb[ts], ps)

        def emit_rope(tb, m):
            """DVE-only: qkr = raw*cos + shuffle(raw)*sin_signed."""
            r = raw_sb.pop((tb, m))
            rot = work.tile([128, TB], BF16, tag="rot")
            nc.vector.stream_shuffle(rot, r, SHUF)
            qc = work.tile([128, TB], BF16, tag="qc")
            nc.vector.tensor_mul(qc, r, cos_sb[:, tb * TB:(tb + 1) * TB])
            rs = work.tile([128, TB], BF16, tag="rs")
            nc.vector.tensor_mul(rs, rot, sin_sb[:, tb * TB:(tb + 1) * TB])
            nc.vector.tensor_add(qk_rope[m][tb], qc, rs)

        def emit_proj_half(qb, s, n, c, ps_override=None):
            if ps_override is not None:
                ps = ps_override
            else:
                h = next_half()
                ps = mm2[:, h, :]
            for p in range(2):
                nc.tensor.matmul(
                    ps, lhsT=outT_sb[(qb, p)][:, s * 128:(s + 1) * 128],
                    rhs=wpr_sb[:, p, n * TB + c * HB:n * TB + (c + 1) * HB],
                    start=(p == 0), stop=(p == 1))
            if (qb, s) not in pr_tiles:
                pr_tiles[(qb, s)] = prp.tile([128, 2, TB], BF16, tag="pr",
                                             name=f"pr{qb}_{s}")
            prt = pr_tiles[(qb, s)]
            nc.vector.tensor_copy(prt[:, n, c * HB:(c + 1) * HB], ps)
            if n == 1 and c == 1:
                nc.gpsimd.dma_start(
                    out=out_d[:, (qb * 4 + s) * DIM:(qb * 4 + s + 1) * DIM],
                    in_=prt.rearrange("p a b -> p (a b)"))

        # ---------- attention pieces ----------
        av_ps = {}
        rsum_ps = {}

        expd = {}

        def emit_scores_exp(ph, kt):
            qb, W0, QW, nkt, db = ph
            ktl = kt - db
            a = 128 * ktl if ktl >= 0 else 0
            w = QW - a
            tbk, ok = kt // 4, (kt % 4) * 128
            for p in range(2):
                sc2 = ps_sc.tile([128, 2, TB], F32, tag="sc",
                                 name=f"sc{qb}_{W0}_{kt}_{p}")
                for j in range(2):
                    nc.tensor.matmul(
                        sc2[:, j, 0:w],
                        lhsT=qk_rope[2 + p][tbk][64 * j:64 * j + 64, ok:ok + 128],
                        rhs=qk_rope[p][qb][64 * j:64 * j + 64, W0 + a:W0 + QW],
                        start=True, stop=True, tile_position=(64 * j, 0),
                    )
                exp2 = expp.tile([128, 2, TB], BF16, tag="exp",
                                 name=f"exp{qb}_{W0}_{kt}_{p}")
                nc.scalar.activation(exp2[:, :, 0:w], sc2[:, :, 0:w],
                                     mybir.ActivationFunctionType.Exp,
                                     scale=float(SCALE))
                if ktl >= 0:
                    nc.vector.tensor_mul(exp2[:, :, 0:128], exp2[:, :, 0:128],
                                         mask_bc)
                expd[(qb, W0, kt, p)] = exp2

        def emit_av_rsum(ph, kt):
            qb, W0, QW, nkt, db = ph
            ktl = kt - db
            a = 128 * ktl if ktl >= 0 else 0
            w = QW - a
            for p in range(2):
                exp2 = expd.pop((qb, W0, kt, p))
                for j in range(2):
                    h = 2 * p + j
                    nc.tensor.matmul(
                        av_ps[(qb, W0, p)][64 * j:64 * j + 64, a:QW],
                        lhsT=v_sb[kt][:, 64 * h:64 * h + 64],
                        rhs=exp2[:, j, 0:w],
                        start=(kt == 0), stop=(kt == nkt - 1),
                        skip_group_check=True,
                        tile_position=(0, 64 * j),
                    )
                for j in range(2):
                    h = 2 * p + j
                    nc.tensor.matmul(
                        rsum_ps[(qb, W0)][32 * h:32 * h + 1, a:QW],
                        lhsT=ones_sb,
                        rhs=exp2[:, j, 0:w],
                        start=(kt == 0), stop=(kt == nkt - 1),
                        skip_group_check=True,
                        tile_position=(0, 32 * h),
                    )

        def emit_norm(ph, c0, c1):
            qb, W0, QW, nkt, db = ph
            rsum_sb = work.tile([128, TB], BF16, tag="recip")
            nc.vector.tensor_copy(rsum_sb[:, c0:c1], rsum_ps[(qb, W0)][:, c0:c1])
            bc2 = ps_sc.tile([128, 2, TB], F32, tag="sc", name=f"bc{qb}_{W0}_{c0}")
            for p in range(2):
                for j in range(2):
                    h = 2 * p + j
                    nc.tensor.matmul(
                        bc2[64 * j:64 * j + 64, p, c0:c1],
                        lhsT=ones2_sb[32 * h:32 * h + 1, :],
                        rhs=rsum_sb[32 * h:32 * h + 1, c0:c1],
                        start=True, stop=True, skip_group_check=True,
                        tile_position=(32 * h, 64 * j),
                    )
            recip2_sb = work.tile([128, 2, TB], F32, tag="recipb")
            nc.vector.reciprocal_approx_fast(out=recip2_sb[:, :, c0:c1],
                                             in_=bc2[:, :, c0:c1])
            for p in range(2):
                if (qb, p) not in outT_sb:
                    outT_sb[(qb, p)] = persist.tile(
                        [128, TB], BF16, tag=f"outT{qb}_{p}", name=f"outT{qb}_{p}")
                nc.vector.tensor_mul(outT_sb[(qb, p)][:, W0 + c0:W0 + c1],
                                     av_ps[(qb, W0, p)][:, c0:c1],
                                     recip2_sb[:, p, c0:c1])

        # ---------- pre-loop: QKV q/k for block 0 + rope(0) ----------
        for m in range(4):
            ps = ps_acc.tile([128, TB], F32, tag=["av0", "av1", "rsum"][m % 3],
                             name=f"qk1_{m}_0")
            for d in range(ND):
                nc.tensor.matmul(ps, lhsT=wqk_chunk(d, m), rhs=x_chunk(0, d),
                                 start=(d == 0), stop=(d == ND - 1))
            r = work.tile([128, TB], BF16, tag=f"raw{m}", name=f"raw{m}_0")
            nc.vector.tensor_copy(r, ps)
            raw_sb[(0, m)] = r
        for m in (0, 2, 1, 3):
            emit_rope(0, m)
        for s in range(4):
            emit_v_half(0, s)

        # ---------- main loop: one-behind pipelined attention ----------
        # phase = (qb, W0, QW, nkt, diag_base); the last block is split into
        # two q-column halves so its norm+proj overlap the second half
        phases = [(tb, 0, TB, 4 * (tb + 1), 4 * tb) for tb in range(NTB)]

        def phase_units(qb, W0):
            units = []
            if W0 == 0 and qb + 1 < NTB:
                for m in range(4):
                    units += [lambda m=m, c=c: emit_qk_half(qb + 1, m, c)
                              for c in range(2)]
                    units += [lambda m=m: emit_rope(qb + 1, m)]
                units += [lambda s=s: emit_v_half(qb + 1, s) for s in range(4)]
            if W0 == 0 and qb - 1 >= 0:
                units += [lambda s=s, n=n, c=c: emit_proj_half(qb - 1, s, n, c)
                          for s in range(4) for n in range(2) for c in range(2)]
            return units

        pending = None           # (phase, kt) with exp done, av/rsum pending
        norm_due = None
        for ph in phases:
            qb, W0, QW, nkt, db = ph
            if W0 == 0 and qb + 2 < NTB:
                xn = xt_pool.tile([128, ND, TB], BF16, tag="x", name=f"x_{qb + 2}")
                nc.sync.dma_start(
                    out=xn.rearrange("p a b -> p (a b)"),
                    in_=x_til[:, (qb + 2) * ND * TB:(qb + 3) * ND * TB])
                xt_sb[qb + 2] = xn

            units = phase_units(qb, W0)
            nu = len(units)
            ui = 0
            av_ps[(qb, W0, 0)] = ps_acc.tile([128, TB], F32, tag="av0",
                                             name=f"av0_{qb}_{W0}")
            av_ps[(qb, W0, 1)] = ps_acc.tile([128, TB], F32, tag="av1",
                                             name=f"av1_{qb}_{W0}")
            rsum_ps[(qb, W0)] = ps_acc.tile([128, TB], F32, tag="rsum",
                                            name=f"rsum_{qb}_{W0}")

            for kt in range(nkt):
                emit_scores_exp(ph, kt)
                if pending is not None:
                    emit_av_rsum(*pending)
                pending = (ph, kt)
                if norm_due is not None:
                    emit_norm(*norm_due)
                    norm_due = None
                if qb == NTB - 1 and kt == nkt - 2:
                    # av/rsum cols [0:256] are final after av(kt-1): norm and
                    # project the first two t-subtiles under the last two kts
                    emit_norm(ph, 0, 256)
                    for s in (0, 1):
                        for n in range(2):
                            for c in range(2):
                                emit_proj_half(qb, s, n, c)
                # hold units back one kt so a fresh phase's norm lands
                # first; front-load on the last (ACT-bound) phase
                if qb == NTB - 1:
                    want = 0 if kt == 0 else min(nu, (kt + 1) * nu * 2 // nkt)
                else:
                    want = kt * nu // max(nkt - 1, 1)
                while ui < want:
                    units[ui]()
                    ui += 1
            while ui < nu:
                units[ui]()
                ui += 1
            emit_av_rsum(*pending)
            pending = None
            norm_due = (ph, 256, TB) if qb == NTB - 1 else (ph, 0, TB)

        emit_norm(*norm_due)

        # ---------- tail: final two t-subtiles, 4-deep psum rotation ----------
        tail_ps = ps_acc.tile([128, 2, HB], F32, tag="av0", name="tail_ps")
        tcnt = [0]
        for s in (2, 3):
            for n in range(2):
                for c in range(2):
                    emit_proj_half(NTB - 1, s, n, c,
                                   ps_override=tail_ps[:, (tcnt[0] // 2) % 2, :]
                                   if tcnt[0] % 2 == 0 else None)
                    tcnt[0] += 1


def shard_inputs(x, w_qkv, w_proj):
    """Full inputs -> list of 8 per-core input maps (pre-tiled layouts)."""
    cosT, sinT2, mask01 = _host_constants()
    x = np.ascontiguousarray(np.asarray(x, dtype=np.float32))
    w_qkv = np.asarray(w_qkv, dtype=np.float32)
    w_proj = np.asarray(w_proj, dtype=np.float32)
    bf = ml_dtypes.bfloat16
    in_maps = []
    for c in range(N_CORES):
        b, g = c // TPG, c % TPG
        xT = x[b].T                                           # [DIM, T]
        xt4 = xT.reshape(ND, 128, NTB, TB)                    # [d, p, tb, j]
        x_til = np.ascontiguousarray(xt4.transpose(1, 2, 0, 3).reshape(128, NTB * ND * TB))
        # permute each head's 64 q/k dims by PERM64 (see SHUF comment)
        pidx = (np.arange(LOC) // HEAD_DIM) * HEAD_DIM + np.array(
            [PERM64[i % HEAD_DIM] for i in range(LOC)])
        wq = w_qkv[:, g * LOC:(g + 1) * LOC][:, pidx]
        wk = w_qkv[:, INNER + g * LOC:INNER + (g + 1) * LOC][:, pidx]
        wv = w_qkv[:, 2 * INNER + g * LOC:2 * INNER + (g + 1) * LOC]
        w_qk = np.concatenate([wq, wk], axis=1)               # [DIM, 512]
        wqk_t = w_qk.reshape(ND, 128, 2 * LOC)                # [d, p, c]
        wqk0 = np.ascontiguousarray(wqk_t[0])
        wqkR = np.ascontiguousarray(wqk_t[1:].transpose(1, 0, 2).reshape(128, (ND - 1) * 2 * LOC))
        wv_t = np.ascontiguousarray(
            wv.reshape(ND, 128, LOC).transpose(1, 0, 2).reshape(128, ND * LOC))
        w_pr = w_proj[g * LOC:(g + 1) * LOC, :]               # [256, DIM]
        wpr_t = np.ascontiguousarray(
            w_pr.reshape(2, 128, DIM).transpose(1, 0, 2).reshape(128, 2 * DIM))
        in_maps.append({
            "x_til": x_til.astype(bf),
            "wqk0": wqk0.astype(bf),
            "wqkR": wqkR.astype(bf),
            "wv_til": wv_t.astype(bf),
            "wpr_til": wpr_t.astype(bf),
            "cosT": cosT.astype(bf),
            "sinT2": sinT2.astype(bf),
            "mask01": mask01.astype(bf),
        })
    return in_maps


_CACHE = {}


def _get_compiled():
    if "nc" not in _CACHE:
        nc = bacc.Bacc("TRN2", target_bir_lowering=False, debug=False,
                       enable_asserts=True, num_devices=N_CORES)
        with tile.TileContext(nc) as tc:
            build_kernel(tc)
        nc.compile()
        _CACHE["nc"] = nc
    return _CACHE["nc"]


def kernel(x, w_qkv, w_proj):
    nc = _get_compiled()
    in_maps = shard_inputs(x, w_qkv, w_proj)
    res = run_bass_kernel_spmd(nc, in_maps, core_ids=list(range(N_CORES)))
    outs = []
    for c in range(N_CORES):
        o = np.asarray(res.results[c]["out"], dtype=np.float32)
        o = o.reshape(128, 16, DIM).transpose(1, 0, 2).reshape(T, DIM)
        outs.append(o)
    full = np.stack([
        np.sum([outs[b * TPG + g] for g in range(TPG)], axis=0, dtype=np.float32)
        for b in range(B)
    ])
    return full.astype(np.float32)


# revision 24
# speedup vs baseline: 1.1871x; 1.0012x over previous
"""Causal self-attention with RoPE on 8 TRN2 NeuronCores.

Sharding: 2 (batch) x 4 (head-group tensor parallel). Core c handles
batch b=c//4 and heads [4g, 4g+4) with g=c%4. Each core computes its
q,k,v projections, RoPE, causal attention (transposed-scores flash
layout), and its partial of the output projection; the host sums the
4 partials per batch (the "all-reduce").

v4: one-stage-pipelined attention (av/rowsum trail scores/exp by one
kt so the in-order PE queue never blocks on the exp semaphore); RoPE
rotation via DVE stream_shuffle with the sign folded into the sin
table (no PE rotation matmuls, no rope PSUM); QKV(tb+1)/proj(tb-1)
emitted as half-PSUM-bank ping-pong units inside attention(tb)'s kt
loop; startup DMAs split per 128-row chunk and issued from four
engine queues in parallel.

Self-contained: hardcodes shapes from the problem spec.
"""
import numpy as np
import ml_dtypes

import concourse.bass as bass
import concourse.mybir as mybir
import concourse.tile as tile
from concourse import bacc
from concourse.bass_utils import run_bass_kernel_spmd

F32 = mybir.dt.float32
BF16 = mybir.dt.bfloat16

B, T, DIM = 2, 2048, 1024
HEADS, HEAD_DIM = 16, 64
INNER = HEADS * HEAD_DIM
ROPE_BASE = 10000.0
N_CORES = 8
TPG = 4                      # tensor-parallel group size (head groups)
HPC = HEADS // TPG           # heads per core = 4
LOC = HPC * HEAD_DIM         # local inner = 256
SCALE = 1.0 / np.sqrt(HEAD_DIM)

TB = 512                     # t block for QKV / q block for attention
NTB = T // TB                # 4
ND = DIM // 128              # 8 contraction chunks
HB = 256                     # half-bank column count for ping-pong units

# stream_shuffle permutes within 32-partition blocks (mask replicated
# across blocks). We reorder each head's 64 q/k dims as
# [0..15, 32..47, 16..31, 48..63] so the rope partner (d <-> d+32) sits
# 16 partitions away inside the same 32-block; the shuffle is then a
# 16-half swap. Scores are invariant to this (same perm on q and k).
SHUF = list(range(16, 32)) + list(range(16))
PERM64 = list(range(16)) + list(range(32, 48)) + list(range(16, 32)) + list(range(48, 64))


def _host_constants():
    inv_freq = 1.0 / (ROPE_BASE ** (np.arange(0, HEAD_DIM, 2, dtype=np.float32) / HEAD_DIM))
    t = np.arange(T, dtype=np.float32)
    freqs = np.outer(t, inv_freq).astype(np.float32)          # [T, 32]
    cos32 = np.cos(freqs).T.astype(np.float32)                # [32, T]
    sin32 = np.sin(freqs).T.astype(np.float32)
    cos64 = np.tile(cos32, (2, 1))                            # [64, T]
    sin64 = np.tile(sin32, (2, 1))
    perm = np.array(PERM64)
    # per-head permuted tables; rotate-half sign folded into sin
    cosP = cos64[perm]                                        # [64, T]
    sgn = np.where(perm < 32, -1.0, 1.0)[:, None]
    sinP = sin64[perm] * sgn
    cosT = np.tile(cosP, (2, 1))                              # [128, T]
    sinT2 = np.tile(sinP, (2, 1))

    # post-exp 0/1 causal mask for the diagonal 128-col block: keep j >= p
    j = np.arange(128)[None, :]
    p = np.arange(128)[:, None]
    mask01 = (j >= p).astype(np.float32)                      # [128, 128]
    return cosT, sinT2, mask01


def build_kernel(tc):
    nc = tc.nc
    x_til = nc.dram_tensor("x_til", [128, NTB * ND * TB], BF16, kind="ExternalInput").ap()
    wqk0_d = nc.dram_tensor("wqk0", [128, 2 * LOC], BF16, kind="ExternalInput").ap()
    wqkR_d = nc.dram_tensor("wqkR", [128, (ND - 1) * 2 * LOC], BF16, kind="ExternalInput").ap()
    wv_d = nc.dram_tensor("wv_til", [128, ND * LOC], BF16, kind="ExternalInput").ap()
    wpr_d = nc.dram_tensor("wpr_til", [128, 2 * DIM], BF16, kind="ExternalInput").ap()
    cosT_d = nc.dram_tensor("cosT", [128, T], BF16, kind="ExternalInput").ap()
    sinT_d = nc.dram_tensor("sinT2", [128, T], BF16, kind="ExternalInput").ap()
    mask_d = nc.dram_tensor("mask01", [128, 128], BF16, kind="ExternalInput").ap()
    out_d = nc.dram_tensor("out", [128, NTB * 4 * DIM], BF16, kind="ExternalOutput").ap()

    with (
        tc.tile_pool(name="const", bufs=1) as const,
        tc.tile_pool(name="xt", bufs=2) as xt_pool,
        tc.tile_pool(name="persist", bufs=1) as persist,
        tc.tile_pool(name="work", bufs=4) as work,
        tc.tile_pool(name="prp", bufs=2) as prp,
        tc.tile_pool(name="expp", bufs=6) as expp,
        tc.tile_pool(name="ps_sc", bufs=2, space="PSUM") as ps_sc,
        tc.tile_pool(name="ps_acc", bufs=1, space="PSUM") as ps_acc,
        tc.tile_pool(name="ps_mm", bufs=1, space="PSUM") as ps_mm,
    ):
        # ---- startup DMAs in need-order: the pre-loop's x(0)/w_qk first
        # (split in chunks across the sync + scalar queues so the d-chains
        # can start as chunks land), everything else queued behind ----
        xt0a = const.tile([128, TB], BF16, tag="xt0a")
        nc.sync.dma_start(out=xt0a, in_=x_til[:, 0:TB])
        wqk0 = const.tile([128, 2 * LOC], BF16, tag="wqk0")
        nc.scalar.dma_start(out=wqk0, in_=wqk0_d)
        wqkR = const.tile([128, (ND - 1) * 2 * LOC], BF16, tag="wqkR")
        xt0b = const.tile([128, (ND - 1) * TB], BF16, tag="xt0b")
        for lo, hi in ((1, 4), (4, 8)):
            nc.sync.dma_start(out=xt0b[:, (lo - 1) * TB:(hi - 1) * TB],
                              in_=x_til[:, lo * TB:hi * TB])
            nc.scalar.dma_start(out=wqkR[:, (lo - 1) * 2 * LOC:(hi - 1) * 2 * LOC],
                                in_=wqkR_d[:, (lo - 1) * 2 * LOC:(hi - 1) * 2 * LOC])

        cos_sb = const.tile([128, T], BF16, tag="cos")
        nc.sync.dma_start(out=cos_sb, in_=cosT_d)
        sin_sb = const.tile([128, T], BF16, tag="sin")
        nc.scalar.dma_start(out=sin_sb, in_=sinT_d)
        wv_sb = const.tile([128, ND, LOC], BF16, tag="wv")
        nc.sync.dma_start(out=wv_sb.rearrange("p a b -> p (a b)"), in_=wv_d)
        mask_sb = const.tile([128, 128], BF16, tag="mask")
        nc.scalar.dma_start(out=mask_sb, in_=mask_d)

        ones_sb = const.tile([128, 1], BF16, tag="ones")
        nc.vector.memset(ones_sb, 1.0)
        ones2_sb = const.tile([128, 64], BF16, tag="ones2")
        nc.vector.memset(ones2_sb, 1.0)
        warm_sb = const.tile([128, 128], BF16, tag="warm")
        nc.vector.memset(warm_sb, 0.0)

        def emit_warm(tag, n):
            wp = ps_sc.tile([128, 2, TB], F32, tag="sc", name=f"warm_{tag}")
            for _ in range(n):
                nc.tensor.matmul(wp[:, 0, 0:128], lhsT=warm_sb, rhs=warm_sb,
                                 start=True, stop=True, skip_group_check=True)

        # warm the PE's HAM clock gate before the pre-loop chains
        emit_warm("boot", 40)


        xt_sb = {}
        x1 = xt_pool.tile([128, ND, TB], BF16, tag="x", name="x_1")
        nc.sync.dma_start(out=x1.rearrange("p a b -> p (a b)"),
                          in_=x_til[:, ND * TB:2 * ND * TB])
        xt_sb[1] = x1
        wpr_sb = const.tile([128, 2, DIM], BF16, tag="wpr")
        nc.scalar.dma_start(out=wpr_sb.rearrange("p a b -> p (a b)"), in_=wpr_d)

        mask_bc = mask_sb.rearrange("p (o n) -> p o n", o=1).to_broadcast([128, 2, 128])

        def x_chunk(tb, d):
            if tb == 0:
                return xt0a if d == 0 else xt0b[:, (d - 1) * TB:d * TB]
            return xt_sb[tb][:, d, :]

        def wqk_chunk(d, m):
            if d == 0:
                return wqk0[:, m * 128:(m + 1) * 128]
            return wqkR[:, (d - 1) * 2 * LOC + m * 128:(d - 1) * 2 * LOC + (m + 1) * 128]

        # persistent per-phase outputs
        qk_rope = [[persist.tile([128, TB], BF16, tag=f"qkr{m}_{tb}", name=f"qkr{m}_{tb}")
                    for tb in range(NTB)] for m in range(4)]
        v_sb = [persist.tile([128, LOC], BF16, tag=f"v{ts}", name=f"v{ts}")
                for ts in range(4 * NTB)]
        raw_sb = {}
        outT_sb = {}
        pr_tiles = {}

        # the single ping-pong PSUM bank for pipelined QKV/v/proj units
        mm2 = ps_mm.tile([128, 2, HB], F32, tag="mm", name="mm2")
        half = [0]

        def next_half():
            h = half[0]
            half[0] ^= 1
            return h

        # ---------- emission units ----------
        def emit_qk_half(tb, m, c):
            """c in {0,1}: column half of the [128, TB] q/k pair output."""
            h = next_half()
            ps = mm2[:, h, :]
            for d in range(ND):
                nc.tensor.matmul(ps, lhsT=wqk_chunk(d, m),
                                 rhs=x_chunk(tb, d)[:, c * HB:(c + 1) * HB],
                                 start=(d == 0), stop=(d == ND - 1))
            if (tb, m) not in raw_sb:
                raw_sb[(tb, m)] = work.tile([128, TB], BF16, tag=f"raw{m}",
                                            name=f"raw{m}_{tb}")
            nc.vector.tensor_copy(raw_sb[(tb, m)][:, c * HB:(c + 1) * HB], ps)

        def emit_v_half(tb, s):
            ts = tb * 4 + s
            h = next_half()
            ps = mm2[:, h, :]
            for d in range(ND):
                nc.tensor.matmul(ps, lhsT=x_chunk(tb, d)[:, s * 128:(s + 1) * 128],
                                 rhs=wv_sb[:, d, :],
                                 start=(d == 0), stop=(d == ND - 1))
            nc.vector.tensor_copy(v_sb[ts], ps)

        def emit_rope(tb, m):
            """DVE-only: qkr = raw*cos + shuffle(raw)*sin_signed."""
            r = raw_sb.pop((tb, m))
            rot = work.tile([128, TB], BF16, tag="rot")
            nc.vector.stream_shuffle(rot, r, SHUF)
            qc = work.tile([128, TB], BF16, tag="qc")
            nc.vector.tensor_mul(qc, r, cos_sb[:, tb * TB:(tb + 1) * TB])
            rs = work.tile([128, TB], BF16, tag="rs")
            nc.vector.tensor_mul(rs, rot, sin_sb[:, tb * TB:(tb + 1) * TB])
            nc.vector.tensor_add(qk_rope[m][tb], qc, rs)

        def emit_proj_half(qb, s, n, c, ps_override=None):
            if ps_override is not None:
                ps = ps_override
            else:
                h = next_half()
                ps = mm2[:, h, :]
            for p in range(2):
                nc.tensor.matmul(
                    ps, lhsT=outT_sb[(qb, p)][:, s * 128:(s + 1) * 128],
                    rhs=wpr_sb[:, p, n * TB + c * HB:n * TB + (c + 1) * HB],
                    start=(p == 0), stop=(p == 1))
            if (qb, s) not in pr_tiles:
                pr_tiles[(qb, s)] = prp.tile([128, 2, TB], BF16, tag="pr",
                                             name=f"pr{qb}_{s}")
            prt = pr_tiles[(qb, s)]
            nc.vector.tensor_copy(prt[:, n, c * HB:(c + 1) * HB], ps)
            if c == 1:
                nc.gpsimd.dma_start(
                    out=out_d[:, (qb * 4 + s) * DIM + n * TB:
                              (qb * 4 + s) * DIM + (n + 1) * TB],
                    in_=prt[:, n, :])

        # ---------- attention pieces ----------
        av_ps = {}
        rsum_ps = {}

        expd = {}

        def emit_scores_exp(ph, kt):
            qb, W0, QW, nkt, db = ph
            ktl = kt - db
            a = 128 * ktl if ktl >= 0 else 0
            w = QW - a
            tbk, ok = kt // 4, (kt % 4) * 128
            for p in range(2):
                sc2 = ps_sc.tile([128, 2, TB], F32, tag="sc",
                                 name=f"sc{qb}_{W0}_{kt}_{p}")
                for j in range(2):
                    nc.tensor.matmul(
                        sc2[:, j, 0:w],
                        lhsT=qk_rope[2 + p][tbk][64 * j:64 * j + 64, ok:ok + 128],
                        rhs=qk_rope[p][qb][64 * j:64 * j + 64, W0 + a:W0 + QW],
                        start=True, stop=True, tile_position=(64 * j, 0),
                    )
                exp2 = expp.tile([128, 2, TB], BF16, tag="exp",
                                 name=f"exp{qb}_{W0}_{kt}_{p}")
                nc.scalar.activation(exp2[:, :, 0:w], sc2[:, :, 0:w],
                                     mybir.ActivationFunctionType.Exp,
                                     scale=float(SCALE))
                if ktl >= 0:
                    nc.vector.tensor_mul(exp2[:, :, 0:128], exp2[:, :, 0:128],
                                         mask_bc)
                expd[(qb, W0, kt, p)] = exp2

        def emit_av_rsum(ph, kt):
            qb, W0, QW, nkt, db = ph
            ktl = kt - db
            a = 128 * ktl if ktl >= 0 else 0
            w = QW - a
            for p in range(2):
                exp2 = expd.pop((qb, W0, kt, p))
                for j in range(2):
                    h = 2 * p + j
                    nc.tensor.matmul(
                        av_ps[(qb, W0, p)][64 * j:64 * j + 64, a:QW],
                        lhsT=v_sb[kt][:, 64 * h:64 * h + 64],
                        rhs=exp2[:, j, 0:w],
                        start=(kt == 0), stop=(kt == nkt - 1),
                        skip_group_check=True,
                        tile_position=(0, 64 * j),
                    )
                for j in range(2):
                    h = 2 * p + j
                    nc.tensor.matmul(
                        rsum_ps[(qb, W0)][32 * h:32 * h + 1, a:QW],
                        lhsT=ones_sb,
                        rhs=exp2[:, j, 0:w],
                        start=(kt == 0), stop=(kt == nkt - 1),
                        skip_group_check=True,
                        tile_position=(0, 32 * h),
                    )

        def emit_norm(ph, c0, c1):
            qb, W0, QW, nkt, db = ph
            rsum_sb = work.tile([128, TB], BF16, tag="recip")
            nc.vector.tensor_copy(rsum_sb[:, c0:c1], rsum_ps[(qb, W0)][:, c0:c1])
            bc2 = ps_sc.tile([128, 2, TB], F32, tag="sc", name=f"bc{qb}_{W0}_{c0}")
            for p in range(2):
                for j in range(2):
                    h = 2 * p + j
                    nc.tensor.matmul(
                        bc2[64 * j:64 * j + 64, p, c0:c1],
                        lhsT=ones2_sb[32 * h:32 * h + 1, :],
                        rhs=rsum_sb[32 * h:32 * h + 1, c0:c1],
                        start=True, stop=True, skip_group_check=True,
                        tile_position=(32 * h, 64 * j),
                    )
            recip2_sb = work.tile([128, 2, TB], F32, tag="recipb")
            nc.vector.reciprocal_approx_fast(out=recip2_sb[:, :, c0:c1],
                                             in_=bc2[:, :, c0:c1])
            for p in range(2):
                if (qb, p) not in outT_sb:
                    outT_sb[(qb, p)] = persist.tile(
                        [128, TB], BF16, tag=f"outT{qb}_{p}", name=f"outT{qb}_{p}")
                nc.vector.tensor_mul(outT_sb[(qb, p)][:, W0 + c0:W0 + c1],
                                     av_ps[(qb, W0, p)][:, c0:c1],
                                     recip2_sb[:, p, c0:c1])

        # ---------- pre-loop: QKV q/k for block 0 + rope(0) ----------
        for m in range(4):
            ps = ps_acc.tile([128, TB], F32, tag=["av0", "av1", "rsum"][m % 3],
                             name=f"qk1_{m}_0")
            for d in range(ND):
                nc.tensor.matmul(ps, lhsT=wqk_chunk(d, m), rhs=x_chunk(0, d),
                                 start=(d == 0), stop=(d == ND - 1))
            r = work.tile([128, TB], BF16, tag=f"raw{m}", name=f"raw{m}_0")
            nc.vector.tensor_copy(r, ps)
            raw_sb[(0, m)] = r
        for m in (0, 2, 1, 3):
            emit_rope(0, m)
        for s in range(4):
            emit_v_half(0, s)

        # ---------- main loop: one-behind pipelined attention ----------
        # phase = (qb, W0, QW, nkt, diag_base); the last block is split into
        # two q-column halves so its norm+proj overlap the second half
        phases = [(tb, 0, TB, 4 * (tb + 1), 4 * tb) for tb in range(NTB)]

        def phase_units(qb, W0):
            units = []
            if W0 == 0 and qb + 1 < NTB:
                for m in range(4):
                    units += [lambda m=m, c=c: emit_qk_half(qb + 1, m, c)
                              for c in range(2)]
                    units += [lambda m=m: emit_rope(qb + 1, m)]
                units += [lambda s=s: emit_v_half(qb + 1, s) for s in range(4)]
            if W0 == 0 and qb - 1 >= 0:
                units += [lambda s=s, n=n, c=c: emit_proj_half(qb - 1, s, n, c)
                          for s in range(4) for n in range(2) for c in range(2)]
            return units

        pending = None           # (phase, kt) with exp done, av/rsum pending
        norm_due = None
        for ph in phases:
            qb, W0, QW, nkt, db = ph
            if W0 == 0 and qb + 2 < NTB:
                xn = xt_pool.tile([128, ND, TB], BF16, tag="x", name=f"x_{qb + 2}")
                nc.sync.dma_start(
                    out=xn.rearrange("p a b -> p (a b)"),
                    in_=x_til[:, (qb + 2) * ND * TB:(qb + 3) * ND * TB])
                xt_sb[qb + 2] = xn

            units = phase_units(qb, W0)
            nu = len(units)
            ui = 0
            av_ps[(qb, W0, 0)] = ps_acc.tile([128, TB], F32, tag="av0",
                                             name=f"av0_{qb}_{W0}")
            av_ps[(qb, W0, 1)] = ps_acc.tile([128, TB], F32, tag="av1",
                                             name=f"av1_{qb}_{W0}")
            rsum_ps[(qb, W0)] = ps_acc.tile([128, TB], F32, tag="rsum",
                                            name=f"rsum_{qb}_{W0}")

            for kt in range(nkt):
                emit_scores_exp(ph, kt)
                if pending is not None:
                    emit_av_rsum(*pending)
                pending = (ph, kt)
                if norm_due is not None:
                    emit_norm(*norm_due)
                    norm_due = None
                if qb == NTB - 1 and kt == nkt - 2:
                    # av/rsum cols [0:256] are final after av(kt-1): norm and
                    # project the first two t-subtiles under the last two kts
                    emit_norm(ph, 0, 256)
                    for s in (0, 1):
                        for n in range(2):
                            for c in range(2):
                                emit_proj_half(qb, s, n, c)
                # hold units back one kt so a fresh phase's norm lands
                # first; front-load on the last (ACT-bound) phase
                if qb == NTB - 1:
                    want = 0 if kt == 0 else min(nu, (kt + 1) * nu * 2 // nkt)
                elif qb == 0:
                    want = min(nu, (kt + 1) * nu // 2)
                else:
                    want = kt * nu // max(nkt - 1, 1)
                while ui < want:
                    units[ui]()
                    ui += 1
            while ui < nu:
                units[ui]()
                ui += 1
            emit_av_rsum(*pending)
            pending = None
            if qb == NTB - 1:
                # keep HAM warm across the norm latency into the tail proj
                emit_warm("tail", 16)
            norm_due = (ph, 256, TB) if qb == NTB - 1 else (ph, 0, TB)

        emit_norm(*norm_due)

        # ---------- tail: final two t-subtiles, 4-deep psum rotation ----------
        tail_ps = ps_acc.tile([128, 2, HB], F32, tag="av0", name="tail_ps")
        tcnt = [0]
        for s in (2, 3):
            for n in range(2):
                for c in range(2):
                    emit_proj_half(NTB - 1, s, n, c,
                                   ps_override=tail_ps[:, (tcnt[0] // 2) % 2, :]
                                   if tcnt[0] % 2 == 0 else None)
                    tcnt[0] += 1


def shard_inputs(x, w_qkv, w_proj):
    """Full inputs -> list of 8 per-core input maps (pre-tiled layouts)."""
    cosT, sinT2, mask01 = _host_constants()
    x = np.ascontiguousarray(np.asarray(x, dtype=np.float32))
    w_qkv = np.asarray(w_qkv, dtype=np.float32)
    w_proj = np.asarray(w_proj, dtype=np.float32)
    bf = ml_dtypes.bfloat16
    in_maps = []
    for c in range(N_CORES):
        b, g = c // TPG, c % TPG
        xT = x[b].T                                           # [DIM, T]
        xt4 = xT.reshape(ND, 128, NTB, TB)                    # [d, p, tb, j]
        x_til = np.ascontiguousarray(xt4.transpose(1, 2, 0, 3).reshape(128, NTB * ND * TB))
        # permute each head's 64 q/k dims by PERM64 (see SHUF comment)
        pidx = (np.arange(LOC) // HEAD_DIM) * HEAD_DIM + np.array(
            [PERM64[i % HEAD_DIM] for i in range(LOC)])
        wq = w_qkv[:, g * LOC:(g + 1) * LOC][:, pidx]
        wk = w_qkv[:, INNER + g * LOC:INNER + (g + 1) * LOC][:, pidx]
        wv = w_qkv[:, 2 * INNER + g * LOC:2 * INNER + (g + 1) * LOC]
        w_qk = np.concatenate([wq, wk], axis=1)               # [DIM, 512]
        wqk_t = w_qk.reshape(ND, 128, 2 * LOC)                # [d, p, c]
        wqk0 = np.ascontiguousarray(wqk_t[0])
        wqkR = np.ascontiguousarray(wqk_t[1:].transpose(1, 0, 2).reshape(128, (ND - 1) * 2 * LOC))
        wv_t = np.ascontiguousarray(
            wv.reshape(ND, 128, LOC).transpose(1, 0, 2).reshape(128, ND * LOC))
        w_pr = w_proj[g * LOC:(g + 1) * LOC, :]               # [256, DIM]
        wpr_t = np.ascontiguousarray(
            w_pr.reshape(2, 128, DIM).transpose(1, 0, 2).reshape(128, 2 * DIM))
        in_maps.append({
            "x_til": x_til.astype(bf),
            "wqk0": wqk0.astype(bf),
            "wqkR": wqkR.astype(bf),
            "wv_til": wv_t.astype(bf),
            "wpr_til": wpr_t.astype(bf),
            "cosT": cosT.astype(bf),
            "sinT2": sinT2.astype(bf),
            "mask01": mask01.astype(bf),
        })
    return in_maps


_CACHE = {}


def _get_compiled():
    if "nc" not in _CACHE:
        nc = bacc.Bacc("TRN2", target_bir_lowering=False, debug=False,
                       enable_asserts=True, num_devices=N_CORES)
        with tile.TileContext(nc) as tc:
            build_kernel(tc)
        nc.compile()
        _CACHE["nc"] = nc
    return _CACHE["nc"]


def kernel(x, w_qkv, w_proj):
    nc = _get_compiled()
    in_maps = shard_inputs(x, w_qkv, w_proj)
    res = run_bass_kernel_spmd(nc, in_maps, core_ids=list(range(N_CORES)))
    outs = []
    for c in range(N_CORES):
        o = np.asarray(res.results[c]["out"], dtype=np.float32)
        o = o.reshape(128, 16, DIM).transpose(1, 0, 2).reshape(T, DIM)
        outs.append(o)
    full = np.stack([
        np.sum([outs[b * TPG + g] for g in range(TPG)], axis=0, dtype=np.float32)
        for b in range(B)
    ])
    return full.astype(np.float32)
